# revision 71
# baseline (speedup 1.0000x reference)
"""Trainium2 Bass kernel for nn_DictNet_44547400794580.

Math: the loss only needs each graph's embedding
    emb_g = (1/N) * (1 - w_g)^T X_g,   w_g = sum_f c_f * (40(L_g - b_f I)^4 + I)^(-2) @ 1
where L_g = I - Ahat_g (sym-normalized Laplacian) and c = C/||C||_2.
All 11 filters are fixed rational functions of Ahat_g (spectrum in [-1,1]); the
combined filter is approximated by ONE degree-11 Chebyshev polynomial (final
loss rel err ~3e-4 host-side vs the 2e-2 gate) evaluated with a
baby-step/giant-step scheme in the product basis T_r(x)*T_q(T_4(x)), r<4, q<3:
  - 2 matrix squarings build T_2, T_4 of Ahat
  - 3 baby vectors via 2 streamed passes (t2 pass carries 2 stationary cols)
  - 2 giant chain steps in T_4 over the 4-column baby block
  - w accumulated by 4 tiny-K matmuls over the row stages
Perf structure:
  - adjacency ships as bf16 with entries 2.0 (exact for a 0/1 matrix; folds
    the 2*Ahat factor), x as bf16; all device matrices are bf16 (PE streams
    bf16/fp32r at the same 1 col/cycle; PSUM accumulates fp32; bf16 storage
    rounding adds ~2e-4 to a 2e-2 loss gate)
  - host pre-arranges adj/x partition-major so every DMA transfer is
    >=2KB-contiguous per partition (small strided segments crawl on the DGEs)
  - bulk DMA on the scalar HW DGE + gpsimd SW DGE queues only (the sync HW
    queue measured ~14GB/s; the scalar queue starves while the scalar engine
    computes, so x rides gpsimd, whose engine idles during the main phase)
  - a ~130-matmul PE warm-up spin on a dedicated tile releases the HAM clock
    gate (1.2 -> 2.4 GHz after ~3.4us sustained) and bridges the DMA/degree
    prologue so the squarings run at full clock
  - every PSUM eviction is a raw same-dtype copy (the DVE fast path; scaled
    or dtype-converting few-partition ops cost ~680ns each) — all stage
    scales are absorbed into the host-side gamma coefficients
  - C-normalization, 1/N, and the constant u-row of the w accumulation fold
    into one scalar-engine [1,N] affine at the end
Sharding: data-parallel over graphs, 2 graphs per core on 8 cores.  The host
gathers the (tiny) [16,256] embeddings and does the final cdist/sparsity
reduction in float64 — the same index bookkeeping the reference itself
performs on the host with numpy.
"""
import sys
if '/opt/trn_rl_repo' not in sys.path:
    sys.path.insert(0, '/opt/trn_rl_repo')

import numpy as np

# ---------------------------------------------------------------------------
# problem constants (hardcoded per contract)
G, N, F, K, NF = 16, 512, 256, 4, 11
NCORES = 8
GPC = G // NCORES          # graphs per core
P = 128
NCH = N // P               # 512 = 4 partition chunks
DEG = 11                   # Chebyshev degree (host rel err ~3e-4 at D=11)
S = 4                      # baby steps
MQ = DEG // S + 1          # giant columns q = 0..2
NG = S * MQ                # 12 product-basis coefficients
NWARM = 130                 # PE warm-up matmuls (~107ns each at cold clock)


# ---------------------------------------------------------------------------
# host-side fixed constants: Chebyshev coefficients of the 11 filters in the
# product basis, as a [NF, NG] matrix (pure math, no input data).
def _build_gamma_mat():
    bs = np.linspace(0.0, 2.0, NF)

    def psi(a, b):
        return (40.0 * (1.0 - a - b) ** 4 + 1.0) ** (-2)

    k = np.arange(DEG + 1)
    xk = np.cos(np.pi * (k + 0.5) / (DEG + 1))
    Mx = np.cos(k[:, None] * np.pi * (k[None, :] + 0.5) / (DEG + 1))

    gm = np.zeros((NF, NG))
    for fi, b in enumerate(bs):
        c = 2.0 / (DEG + 1) * (Mx @ psi(xk, b))
        c[0] *= 0.5
        beta = c.copy()
        gamma = np.zeros((S, MQ))
        for kk in range(DEG, S - 1, -1):
            q, r = divmod(kk, S)
            if r == 0:
                gamma[0, q] = beta[kk]
            else:
                gamma[r, q] = 2.0 * beta[kk]
                beta[S * q - r] -= beta[kk]
        for r in range(S):
            gamma[r, 0] += beta[r]
        # the device stores every stage RAW (pure PSUM-copy evictions, which
        # hit the DVE fast path; scaled [few-partition] tensor_scalar ops
        # cost ~680ns each).  Stored bases:
        #   col1 = 2*T1u (ah2@u), col2 = T2u, col3 = T3u+T1u (t2@col1)
        #   Z1'' = t4d@G (= 2*T1(W)G), Z2'' = t4d@Z1'' (= 2*(T2(W)+I)G)
        gamma[1, :] = (gamma[1, :] - gamma[3, :]) / 2.0
        gamma[:, 0] -= gamma[:, 2]
        gamma[:, 1] /= 2.0
        gamma[:, 2] /= 2.0
        # flatten q-major: index q*S + r
        gm[fi] = gamma.T.reshape(-1)
    return gm.astype(np.float32)


GAMMA_MAT = _build_gamma_mat()          # [11, 12]

TRACE = False
LAST_EXEC_NS = None
LAST_RESULTS = None


# ---------------------------------------------------------------------------
# device kernel (one core: GPC graphs)
def build_device_kernel(tc, outs, ins):
    import concourse.mybir as mybir
    from concourse.masks import make_identity
    from contextlib import ExitStack

    nc = tc.nc
    dt = mybir.dt.float32
    dtr = mybir.dt.float32r
    dtb = mybir.dt.bfloat16
    Alu = mybir.AluOpType

    def mmr(out, lhsT, rhs, **kw):
        nc.tensor.matmul(out, lhsT=lhsT.bitcast(dtr), rhs=rhs.bitcast(dtr), **kw)

    adj_d, x_d, c_d, g_d = ins
    emb_d = outs

    with ExitStack() as ctx:
        ctx.enter_context(nc.allow_low_precision(
            reason="bf16 matrices are intentional: PSUM accumulates fp32, "
                   "bf16 storage rounding adds ~1e-4 to a 2e-2 gate"))
        sb = ctx.enter_context(tc.tile_pool(name="sb", bufs=1))

        adj0 = {}
        xs = {}
        for g in range(GPC):
            adj0[g] = sb.tile([P, NCH, N], dtb, tag=f"adj0_{g}", name=f"adj0_{g}")
            xs[g] = sb.tile([P, NCH, F], dtb, tag=f"xin_{g}", name=f"xin_{g}")

        # warm-up source: first vector-engine op, no other dependencies
        wtile = sb.tile([P, P], dtb, tag="wtile", name="wtile")
        nc.vector.memset(wtile, 0.5)

        # the host pre-arranges adj and x into partition-major layout, so
        # every transfer is >=2KB-contiguous per partition (small strided
        # segments crawl on both the HW and SW DGE queues)
        def adj_half(g, h):
            return adj_d[g][:, h * 2 * N:(h + 1) * 2 * N].rearrange(
                "p (c n) -> p c n", n=N)

        # identity first on gpsimd (the DVE constant chain hangs off it),
        # then that queue's DMA issues
        identg = sb.tile([P, P], dt, tag="identg", name="identg")
        make_identity(nc, identg)

        cvec = sb.tile([NF, 1], dt, tag="cvec", name="cvec")
        gmat = sb.tile([NF, NG], dt, tag="gmat", name="gmat")
        # x rides the gpsimd queue only: the scalar HW queue starves while
        # the scalar engine runs evictions (port contention), gpsimd idles
        nc.scalar.dma_start(adj0[0][:, 0:2, :], adj_half(0, 0))
        nc.gpsimd.dma_start(adj0[0][:, 2:4, :], adj_half(0, 1))
        nc.scalar.dma_start(adj0[1][:, 0:2, :], adj_half(1, 0))
        nc.gpsimd.dma_start(adj0[1][:, 2:4, :], adj_half(1, 1))
        nc.gpsimd.dma_start(xs[1], x_d[1].rearrange("p (c f) -> p c f", f=F))
        nc.gpsimd.dma_start(xs[0], x_d[0].rearrange("p (c f) -> p c f", f=F))
        # tiny constants ride the (slow but sufficient) sync queue so the
        # scalar engine queue is free for its sqrt after just 2 DMA issues
        nc.sync.dma_start(cvec, c_d)
        nc.sync.dma_start(gmat, g_d)

        # ---- PE warm-up spin (HAM clock gate releases after ~3.4us busy)
        with tc.tile_pool(name="pwm", bufs=1, space="PSUM") as pwm:
            ps_warm = pwm.tile([P, P], dt, tag="warm", name="warm")
            for _ in range(NWARM):
                nc.tensor.matmul(ps_warm, lhsT=wtile, rhs=wtile, start=True, stop=True)

        # ---- constants
        identb = sb.tile([P, P], dtb, tag="identb", name="identb")
        nc.vector.tensor_copy(identb, identg)
        negIb = sb.tile([P, P], dtb, tag="negIb", name="negIb")
        nc.vector.tensor_scalar_mul(negIb, identg, -1.0)
        negI2b = sb.tile([P, P], dtb, tag="negI2b", name="negI2b")
        nc.vector.tensor_scalar_mul(negI2b, identg, -2.0)
        ones11 = sb.tile([NF, 1], dt, tag="ones11", name="ones11")
        nc.vector.memset(ones11, 1.0)


        dinv_row = {}
        d2row = {}
        ah2 = {}
        t2 = {}
        t4d = {}
        for g in range(GPC):
            ah2[g] = sb.tile([P, NCH, N], dtb, tag=f"ah{g}", name=f"ah{g}")
            t2[g] = sb.tile([P, NCH, N], dtb, tag=f"t2{g}", name=f"t2{g}")
            t4d[g] = sb.tile([P, NCH, N], dtb, tag=f"t4{g}", name=f"t4{g}")

        with tc.tile_pool(name="psb", bufs=3, space="PSUM") as psb, \
             tc.tile_pool(name="psv", bufs=2, space="PSUM") as psv, \
             tc.tile_pool(name="psx", bufs=1, space="PSUM") as psx:

            def prep_graph(g):
                # adjacency entries arrive as 2.0 (host-folded factor), so the
                # reduce gives 2*deg.  All elementwise work happens in column
                # layout on 128 DVE lanes ([1,N] single-partition DVE ops are
                # ~20x slower — a [1,512] reciprocal measured 3.3us).
                degc = sb.tile([P, NCH], dt, tag=f"degc{g}", name=f"degc{g}")
                nc.vector.tensor_reduce(degc.rearrange("p (c one) -> p c one", one=1),
                                        adj0[g], axis=mybir.AxisListType.X, op=Alu.add)
                dmaxc = sb.tile([P, NCH], dt, tag=f"dmaxc{g}", name=f"dmaxc{g}")
                nc.vector.tensor_scalar(dmaxc, degc, 0.5, 1.0, Alu.mult, Alu.max)
                srootc = sb.tile([P, NCH], dt, tag=f"srootc{g}", name=f"srootc{g}")
                nc.scalar.sqrt(srootc, dmaxc)
                dinvc = sb.tile([P, NCH], dt, tag=f"dinvc{g}", name=f"dinvc{g}")
                nc.vector.reciprocal(dinvc, srootc)
                pscr = psv.tile([S, N], dt, tag="cr", name="cr")[:1, :]
                for kk in range(NCH):
                    nc.tensor.transpose(pscr[:, kk * P:(kk + 1) * P],
                                        dinvc[:, kk:kk + 1], identg)
                dinv_row[g] = sb.tile([1, N], dtb, tag=f"dinv{g}", name=f"dinv{g}")
                nc.vector.tensor_copy(dinv_row[g], pscr)
                # ah2 = 2*Ahat: bf16 rank-1 outer product, masked by adj (=2)
                for kk in range(NCH):
                    dps = psb.tile([P, N], dt, tag="big", name="big")
                    nc.tensor.matmul(dps, lhsT=dinv_row[g][:, kk * P:(kk + 1) * P],
                                     rhs=dinv_row[g], start=True, stop=True)
                    nc.vector.tensor_tensor(ah2[g][:, kk, :],
                                            adj0[g][:, kk, :], dps, Alu.mult)

            # squarings: T2 = (ah2@ah2)/2 - I ; t4d = 4*T2@T2 - 2I (all bf16)
            def square_into(src_m, dst_map, g, scale, dI):
                for m in range(NCH):
                    ps = psb.tile([P, N], dt, tag="big", name="big")
                    for kk in range(NCH):
                        nc.tensor.matmul(ps, lhsT=src_m[g][:, kk, m * P:(m + 1) * P],
                                         rhs=src_m[g][:, kk, :],
                                         start=(kk == 0), stop=(kk == NCH - 1))
                    t = dst_map[g]
                    h = N // 2
                    nc.vector.tensor_scalar_mul(t[:, m, :h], ps[:, :h], scale)
                    nc.scalar.mul(t[:, m, h:], ps[:, h:], scale)
                    nc.vector.tensor_add(t[:, m, m * P:(m + 1) * P],
                                         t[:, m, m * P:(m + 1) * P], dI)

            # prep(1) is emitted after T2(0) so its slow DVE reduce chain
            # overlaps the first squaring instead of blocking its start
            prep_graph(0)
            square_into(ah2, t2, 0, 0.5, negIb)
            prep_graph(1)
            square_into(ah2, t2, 1, 0.5, negIb)
            square_into(t2, t4d, 0, 4.0, negI2b)
            square_into(t2, t4d, 1, 4.0, negI2b)

            # ---- gamma tiles (unnormalized, bf16), nnr = -(1/||C||)/N, and
            # c1 = (1 - rnorm*gamma00)/N folded from the u-row.  Emitted after
            # the squarings: only needed by the w stage.
            gam = {}
            nnr = sb.tile([1, 1], dt, tag="nnr", name="nnr")
            c1s = sb.tile([1, 1], dt, tag="c1s", name="c1s")
            csq = sb.tile([NF, 1], dt, tag="csq", name="csq")
            nc.vector.tensor_mul(csq, cvec, cvec)
            ps1 = psv.tile([S, N], dt, tag="cr", name="cr")[:1, :1]
            nc.tensor.matmul(ps1, lhsT=csq, rhs=ones11, start=True, stop=True)
            snorm = sb.tile([1, 1], dt, tag="snorm", name="snorm")
            nc.scalar.sqrt(snorm, ps1)
            rnorm = sb.tile([1, 1], dt, tag="rnorm", name="rnorm")
            nc.vector.reciprocal(rnorm, snorm)
            nc.vector.tensor_scalar_mul(nnr, rnorm, -1.0 / N)
            # slices of the 12 flat coefficients: [c00 | c01 | c02 c03 | q1 | q2]
            gam00f = sb.tile([1, 1], dt, tag="gam00f", name="gam00f")
            for key, lo, hi in (("c00", 0, 1), ("c01", 1, 2), ("c023", 2, 4),
                                ("q1", 4, 8), ("q2", 8, 12)):
                psq = psv.tile([S, N], dt, tag="cr", name="cr")[:hi - lo, :1]
                nc.tensor.matmul(psq, lhsT=gmat[:, lo:hi], rhs=cvec,
                                 start=True, stop=True)
                if key == "c00":
                    nc.vector.tensor_copy(gam00f, psq)
                else:
                    gam[key] = sb.tile([hi - lo, 1], dtb, tag=f"gam_{key}",
                                       name=f"gam_{key}")
                    nc.vector.tensor_copy(gam[key], psq)
            tt = sb.tile([1, 1], dt, tag="tt", name="tt")
            nc.vector.tensor_mul(tt, rnorm, gam00f)
            nc.vector.tensor_scalar(c1s, tt, -1.0 / N, 1.0 / N, Alu.mult, Alu.add)

            # ---- baby vectors + giant chain (bf16 storage, fp32 PSUM)
            gcol = {}
            z1col = {}
            for g in range(GPC):
                gcol[g] = sb.tile([P, NCH, S], dtb, tag=f"gc{g}", name=f"gc{g}")
                nc.gpsimd.memset(gcol[g][:, :, 0:1], 1.0)
                z1col[g] = sb.tile([P, NCH, S], dtb, tag=f"zc{g}", name=f"zc{g}")

            # PSUM evictions alternate engines per graph so the two
            # graphs' chains drain in parallel (each PSUM-touching DVE/ACT op
            # costs ~150-700ns serially on its engine)
            def ev_copy(g, out, in_):
                nc.vector.tensor_copy(out, in_)

            r1 = {}
            r23 = {}
            z1row = {}
            z2row = {}
            # babies pass 1: g1 = (ah2 @ 1)/2
            onesb = sb.tile([P, 1], dtb, tag="onesb", name="onesb")
            nc.vector.memset(onesb, 1.0)
            for g in range(GPC):
                r1[g] = sb.tile([1, N], dtb, tag=f"r1{g}", name=f"r1{g}")
                ps = psv.tile([S, N], dt, tag="cr", name="cr")[:1, :]
                for kk in range(NCH):
                    nc.tensor.matmul(ps, lhsT=onesb, rhs=ah2[g][:, kk, :],
                                     start=(kk == 0), stop=(kk == NCH - 1))
                ev_copy(g, r1[g], ps)
            # transpose g1 row -> gcol col 1 (stride-2 slots keep PSUM 4B-aligned)
            for g in range(GPC):
                pst = psv.tile([P, NCH * S], dtb, tag="tp", name="tp")[:, :NCH * 2]
                for kk in range(NCH):
                    nc.tensor.transpose(pst[:, kk * 2:kk * 2 + 1],
                                        r1[g][:, kk * P:(kk + 1) * P], identb[:1, :1])
                ev_copy(g, gcol[g][:, :, 1:2],
                        pst.rearrange("p (c two) -> p c two", two=2)[:, :, 0:1])
            # babies pass 2: stream t2 with stationary [u, g1]:
            #   row0 = T2@u = g2 ; row1 = T2@T1@u = h3 (raw; gamma absorbs)
            for g in range(GPC):
                r23[g] = sb.tile([2, N], dtb, tag=f"r23{g}", name=f"r23{g}")
                ps = psv.tile([S, N], dt, tag="cr", name="cr")[:2, :]
                for kk in range(NCH):
                    nc.tensor.matmul(ps, lhsT=gcol[g][:, kk, 0:2], rhs=t2[g][:, kk, :],
                                     start=(kk == 0), stop=(kk == NCH - 1))
                ev_copy(g, r23[g], ps)
            # transpose g2,g3 rows -> gcol cols 2,3
            for g in range(GPC):
                pst = psv.tile([P, NCH * S], dtb, tag="tp", name="tp")[:, :NCH * 2]
                for kk in range(NCH):
                    nc.tensor.transpose(pst[:, kk * 2:(kk + 1) * 2],
                                        r23[g][:, kk * P:(kk + 1) * P], identb[:2, :2])
                ev_copy(g, gcol[g][:, :, 2:4],
                        pst.rearrange("p (c s) -> p c s", s=2))

            # chain step 1: Z1 = T4 @ G   (= t4d@G / 2)
            for g in range(GPC):
                z1row[g] = sb.tile([S, N], dtb, tag=f"z1r{g}", name=f"z1r{g}")
                ps = psv.tile([S, N], dt, tag="cr", name="cr")
                for kk in range(NCH):
                    nc.tensor.matmul(ps, lhsT=gcol[g][:, kk, :], rhs=t4d[g][:, kk, :],
                                     start=(kk == 0), stop=(kk == NCH - 1))
                ev_copy(g, z1row[g], ps)
            for g in range(GPC):
                pst = psv.tile([P, NCH * S], dtb, tag="tp", name="tp")
                for kk in range(NCH):
                    nc.tensor.transpose(pst[:, kk * S:(kk + 1) * S],
                                        z1row[g][:, kk * P:(kk + 1) * P], identb[:S, :S])
                ev_copy(g, z1col[g].rearrange("p c s -> p (c s)"), pst)
            # ---- w accumulation starts EARLY: the first 3 of 4 matmuls only
            # need r1/r23/z1row, so they run before the z2 chain pass instead
            # of serializing after it.  The held wps accumulators occupy both
            # "cr" buffers, so the z2 passes get their own 1-buffer pool
            # (8th PSUM bank).
            wps = {}
            for g in range(GPC):
                wps[g] = psv.tile([S, N], dt, tag="cr", name="cr")[:1, :]
                nc.tensor.matmul(wps[g], lhsT=gam["c01"], rhs=r1[g],
                                 start=True, stop=False, skip_group_check=True)
                nc.tensor.matmul(wps[g], lhsT=gam["c023"], rhs=r23[g],
                                 start=False, stop=False, skip_group_check=True)
                nc.tensor.matmul(wps[g], lhsT=gam["q1"], rhs=z1row[g],
                                 start=False, stop=False, skip_group_check=True)
            # chain step 2: Z2' = t4d@Z1 (raw; gamma absorbs the -G term)
            for g in range(GPC):
                z2row[g] = sb.tile([S, N], dtb, tag=f"z2r{g}", name=f"z2r{g}")
                ps = psx.tile([S, N], dt, tag="cz", name="cz")
                for kk in range(NCH):
                    nc.tensor.matmul(ps, lhsT=z1col[g][:, kk, :], rhs=t4d[g][:, kk, :],
                                     start=(kk == 0), stop=(kk == NCH - 1))
                ev_copy(g, z2row[g], ps)

            #     w = c01*g1 + c023^T r23 + q1^T Z1 + q2^T Z2'
            #     v = c1s + nnr*w ; emb = v^T X (bf16)
            vrow = {}
            vcol = {}
            for g in range(GPC):
                nc.tensor.matmul(wps[g], lhsT=gam["q2"], rhs=z2row[g],
                                 start=False, stop=True, skip_group_check=True)
                vrow[g] = sb.tile([1, N], dtb, tag=f"vrow{g}", name=f"vrow{g}")
                nc.scalar.activation(vrow[g], wps[g],
                                     mybir.ActivationFunctionType.Identity,
                                     bias=c1s[:, 0:1], scale=nnr[:, 0:1])
            for g in range(GPC):
                pst = psv.tile([P, NCH * S], dtb, tag="tp", name="tp")[:, :NCH * 2]
                for kk in range(NCH):
                    nc.tensor.transpose(pst[:, kk * 2:kk * 2 + 1],
                                        vrow[g][:, kk * P:(kk + 1) * P], identb[:1, :1])
                vcol[g] = sb.tile([P, NCH], dtb, tag=f"vc{g}", name=f"vc{g}")
                ev_copy(g, vcol[g],
                        pst.rearrange("p (c two) -> p c two", two=2)[:, :, 0])
            for g in range(GPC):
                pse = psv.tile([S, N], dt, tag="cr", name="cr")[:1, :F]
                for kk in range(NCH):
                    nc.tensor.matmul(pse, lhsT=vcol[g][:, kk:kk + 1],
                                     rhs=xs[g][:, kk, :],
                                     start=(kk == 0), stop=(kk == NCH - 1))
                erow = sb.tile([1, F], dt, tag=f"erow{g}", name=f"erow{g}")
                ev_copy(g, erow, pse)
                nc.scalar.dma_start(emb_d[g:g + 1, :], erow)


# ---------------------------------------------------------------------------
# host: final loss from embeddings (float64; same bookkeeping the reference
# does on the host with numpy: class index construction / product combos)
def final_loss(emb, C, y):
    from itertools import product as _product
    e = emb.astype(np.float64)
    sq = (e * e).sum(1)
    D2 = sq[:, None] + sq[None, :] - 2 * e @ e.T
    D = np.sqrt(np.maximum(D2, 0.0))
    np.fill_diagonal(D, 0.0)
    y = np.asarray(y)
    class_idx = [np.nonzero(y == i)[0] for i in range(K)]
    neg = np.array(list(_product(*class_idx)))
    h1 = -sum(D[np.ix_(cb, cb)].mean() for cb in neg)
    h2 = sum(D[np.ix_(ci, ci)].mean() for ci in class_idx)
    beta = neg.shape[0] / K
    C64 = np.asarray(C, np.float64)
    dims = np.sqrt(float(C64.shape[0]))
    l1 = np.abs(C64).sum(0)
    l2 = np.sqrt((C64 * C64).sum(0))
    sparsity = np.mean((dims - l1 / l2) / (dims - 1))
    return sparsity + h2 + h1 / beta


# ---------------------------------------------------------------------------
_COMPILED = {}


def _get_nc():
    if "nc" in _COMPILED:
        return _COMPILED["nc"]
    import concourse.mybir as mybir
    import concourse.tile as tile
    from concourse import bacc

    dt = mybir.dt.float32
    nc = bacc.Bacc("TRN2", target_bir_lowering=False, debug=False)
    adj_d = nc.dram_tensor("adj", [GPC, P, NCH * N], mybir.dt.bfloat16,
                           kind="ExternalInput").ap()
    x_d = nc.dram_tensor("x", [GPC, P, NCH * F], mybir.dt.bfloat16,
                         kind="ExternalInput").ap()
    c_d = nc.dram_tensor("cvec", [NF, 1], dt, kind="ExternalInput").ap()
    g_d = nc.dram_tensor("gmat", [NF, NG], dt, kind="ExternalInput").ap()
    emb_d = nc.dram_tensor("emb", [GPC, F], dt, kind="ExternalOutput").ap()

    with tile.TileContext(nc) as tc:
        build_device_kernel(tc, emb_d, (adj_d, x_d, c_d, g_d))
    nc.compile()

    _COMPILED["nc"] = nc
    return nc


def kernel(adj, x, C, y):
    global LAST_EXEC_NS, LAST_RESULTS
    from concourse.bass_utils import run_bass_kernel_spmd
    import ml_dtypes

    # adjacency ships as bf16 with entries 2.0 (exact): folds the 2*Ahat
    # factor into the mask multiply; x tolerates bf16 (the emb mean averages
    # the rounding noise far below the accuracy gate).  Both are pre-arranged
    # partition-major ([g, p, chunk*inner]) so device DMAs are contiguous.
    adj = (np.asarray(adj, np.float32) * 2.0).astype(ml_dtypes.bfloat16)
    adj = np.ascontiguousarray(
        adj.reshape(G, NCH, P, N).transpose(0, 2, 1, 3).reshape(G, P, NCH * N))
    x = np.asarray(x, np.float32).astype(ml_dtypes.bfloat16)
    x = np.ascontiguousarray(
        x.reshape(G, NCH, P, F).transpose(0, 2, 1, 3).reshape(G, P, NCH * F))
    C = np.ascontiguousarray(np.asarray(C, np.float32))

    nc = _get_nc()
    in_maps = []
    for c in range(NCORES):
        in_maps.append({
            "adj": adj[c * GPC:(c + 1) * GPC],
            "x": x[c * GPC:(c + 1) * GPC],
            "cvec": C,
            "gmat": GAMMA_MAT,
        })
    import time as _time
    for attempt in range(3):
        try:
            res = run_bass_kernel_spmd(nc, in_maps, core_ids=list(range(NCORES)), trace=TRACE)
            break
        except Exception:
            # transient device errors (e.g. NRT_EXEC_UNIT_UNRECOVERABLE from a
            # previously killed process) clear after a moment
            if attempt == 2:
                raise
            _time.sleep(2.0)
    LAST_EXEC_NS = res.exec_time_ns
    LAST_RESULTS = res
    emb = np.concatenate([res.results[c]["emb"] for c in range(NCORES)], axis=0)
    loss = final_loss(emb, C, y)
    return np.float32(loss)


# revision 72
# speedup vs baseline: 1.0166x; 1.0166x over previous
"""Trainium2 Bass kernel for nn_DictNet_44547400794580.

Math: the loss only needs each graph's embedding
    emb_g = (1/N) * (1 - w_g)^T X_g,   w_g = sum_f c_f * (40(L_g - b_f I)^4 + I)^(-2) @ 1
where L_g = I - Ahat_g (sym-normalized Laplacian) and c = C/||C||_2.
All 11 filters are fixed rational functions of Ahat_g (spectrum in [-1,1]); the
combined filter is approximated by ONE degree-11 Chebyshev polynomial (final
loss rel err ~3e-4 host-side vs the 2e-2 gate) evaluated with a
baby-step/giant-step scheme in the product basis T_r(x)*T_q(T_4(x)), r<4, q<3:
  - 2 matrix squarings build T_2, T_4 of Ahat
  - 3 baby vectors via 2 streamed passes (t2 pass carries 2 stationary cols)
  - 2 giant chain steps in T_4 over the 4-column baby block
  - w accumulated by 4 tiny-K matmuls over the row stages
Perf structure:
  - adjacency ships as bf16 with entries 2.0 (exact for a 0/1 matrix; folds
    the 2*Ahat factor), x as bf16; all device matrices are bf16 (PE streams
    bf16/fp32r at the same 1 col/cycle; PSUM accumulates fp32; bf16 storage
    rounding adds ~2e-4 to a 2e-2 loss gate)
  - host pre-arranges adj/x partition-major so every DMA transfer is
    >=2KB-contiguous per partition (small strided segments crawl on the DGEs)
  - bulk DMA on the scalar HW DGE + gpsimd SW DGE queues only (the sync HW
    queue measured ~14GB/s; the scalar queue starves while the scalar engine
    computes, so x rides gpsimd, whose engine idles during the main phase)
  - a ~130-matmul PE warm-up spin on a dedicated tile releases the HAM clock
    gate (1.2 -> 2.4 GHz after ~3.4us sustained) and bridges the DMA/degree
    prologue so the squarings run at full clock
  - every PSUM eviction is a raw same-dtype copy (the DVE fast path; scaled
    or dtype-converting few-partition ops cost ~680ns each) — all stage
    scales are absorbed into the host-side gamma coefficients
  - C-normalization, 1/N, and the constant u-row of the w accumulation fold
    into one scalar-engine [1,N] affine at the end
Sharding: data-parallel over graphs, 2 graphs per core on 8 cores.  The host
gathers the (tiny) [16,256] embeddings and does the final cdist/sparsity
reduction in float64 — the same index bookkeeping the reference itself
performs on the host with numpy.
"""
import sys
if '/opt/trn_rl_repo' not in sys.path:
    sys.path.insert(0, '/opt/trn_rl_repo')

import numpy as np

# ---------------------------------------------------------------------------
# problem constants (hardcoded per contract)
G, N, F, K, NF = 16, 512, 256, 4, 11
NCORES = 8
GPC = G // NCORES          # graphs per core
P = 128
NCH = N // P               # 512 = 4 partition chunks
DEG = 11                   # Chebyshev degree (host rel err ~3e-4 at D=11)
S = 4                      # baby steps
MQ = DEG // S + 1          # giant columns q = 0..2
NG = S * MQ                # 12 product-basis coefficients
NWARM = 130                 # PE warm-up matmuls (~107ns each at cold clock)


# ---------------------------------------------------------------------------
# host-side fixed constants: Chebyshev coefficients of the 11 filters in the
# product basis, as a [NF, NG] matrix (pure math, no input data).
def _build_gamma_mat():
    bs = np.linspace(0.0, 2.0, NF)

    def psi(a, b):
        return (40.0 * (1.0 - a - b) ** 4 + 1.0) ** (-2)

    k = np.arange(DEG + 1)
    xk = np.cos(np.pi * (k + 0.5) / (DEG + 1))
    Mx = np.cos(k[:, None] * np.pi * (k[None, :] + 0.5) / (DEG + 1))

    gm = np.zeros((NF, NG))
    for fi, b in enumerate(bs):
        c = 2.0 / (DEG + 1) * (Mx @ psi(xk, b))
        c[0] *= 0.5
        beta = c.copy()
        gamma = np.zeros((S, MQ))
        for kk in range(DEG, S - 1, -1):
            q, r = divmod(kk, S)
            if r == 0:
                gamma[0, q] = beta[kk]
            else:
                gamma[r, q] = 2.0 * beta[kk]
                beta[S * q - r] -= beta[kk]
        for r in range(S):
            gamma[r, 0] += beta[r]
        # the device stores every stage RAW (pure PSUM-copy evictions, which
        # hit the DVE fast path; scaled [few-partition] tensor_scalar ops
        # cost ~680ns each).  Stored bases:
        #   col1 = 2*T1u (ah2@u), col2 = T2u, col3 = T3u+T1u (t2@col1)
        #   Z1'' = t4d@G (= 2*T1(W)G), Z2'' = t4d@Z1'' (= 2*(T2(W)+I)G)
        gamma[1, :] = (gamma[1, :] - gamma[3, :]) / 2.0
        gamma[:, 0] -= gamma[:, 2]
        gamma[:, 1] /= 2.0
        gamma[:, 2] /= 2.0
        # flatten q-major: index q*S + r
        gm[fi] = gamma.T.reshape(-1)
    return gm.astype(np.float32)


GAMMA_MAT = _build_gamma_mat()          # [11, 12]

TRACE = False
LAST_EXEC_NS = None
LAST_RESULTS = None


# ---------------------------------------------------------------------------
# device kernel (one core: GPC graphs)
def build_device_kernel(tc, outs, ins):
    import concourse.mybir as mybir
    from concourse.masks import make_identity
    from contextlib import ExitStack

    nc = tc.nc
    dt = mybir.dt.float32
    dtr = mybir.dt.float32r
    dtb = mybir.dt.bfloat16
    Alu = mybir.AluOpType

    def mmr(out, lhsT, rhs, **kw):
        nc.tensor.matmul(out, lhsT=lhsT.bitcast(dtr), rhs=rhs.bitcast(dtr), **kw)

    adj_d, x_d, c_d, g_d = ins
    emb_d = outs

    with ExitStack() as ctx:
        ctx.enter_context(nc.allow_low_precision(
            reason="bf16 matrices are intentional: PSUM accumulates fp32, "
                   "bf16 storage rounding adds ~1e-4 to a 2e-2 gate"))
        sb = ctx.enter_context(tc.tile_pool(name="sb", bufs=1))

        adj0 = {}
        xs = {}
        for g in range(GPC):
            adj0[g] = sb.tile([P, NCH, N], dtb, tag=f"adj0_{g}", name=f"adj0_{g}")
            xs[g] = sb.tile([P, NCH, F], dtb, tag=f"xin_{g}", name=f"xin_{g}")

        # warm-up source: first vector-engine op, no other dependencies
        wtile = sb.tile([P, P], dtb, tag="wtile", name="wtile")
        nc.vector.memset(wtile, 0.5)

        # the host pre-arranges adj and x into partition-major layout, so
        # every transfer is >=2KB-contiguous per partition (small strided
        # segments crawl on both the HW and SW DGE queues)
        def adj_half(g, h):
            return adj_d[g][:, h * 2 * N:(h + 1) * 2 * N].rearrange(
                "p (c n) -> p c n", n=N)

        # identity first on gpsimd (the DVE constant chain hangs off it),
        # then that queue's DMA issues
        identg = sb.tile([P, P], dt, tag="identg", name="identg")
        make_identity(nc, identg)

        cvec = sb.tile([NF, 1], dt, tag="cvec", name="cvec")
        gmat = sb.tile([NF, NG], dt, tag="gmat", name="gmat")
        # x rides the gpsimd queue only: the scalar HW queue starves while
        # the scalar engine runs evictions (port contention), gpsimd idles
        nc.scalar.dma_start(adj0[0][:, 0:2, :], adj_half(0, 0))
        nc.gpsimd.dma_start(adj0[0][:, 2:4, :], adj_half(0, 1))
        nc.scalar.dma_start(adj0[1][:, 0:2, :], adj_half(1, 0))
        nc.gpsimd.dma_start(adj0[1][:, 2:4, :], adj_half(1, 1))
        nc.gpsimd.dma_start(xs[1], x_d[1].rearrange("p (c f) -> p c f", f=F))
        nc.gpsimd.dma_start(xs[0], x_d[0].rearrange("p (c f) -> p c f", f=F))
        # tiny constants ride the (slow but sufficient) sync queue so the
        # scalar engine queue is free for its sqrt after just 2 DMA issues
        nc.sync.dma_start(cvec, c_d)
        nc.sync.dma_start(gmat, g_d)

        # ---- PE warm-up spin (HAM clock gate releases after ~3.4us busy)
        with tc.tile_pool(name="pwm", bufs=1, space="PSUM") as pwm:
            ps_warm = pwm.tile([P, P], dt, tag="warm", name="warm")
            for _ in range(NWARM):
                nc.tensor.matmul(ps_warm, lhsT=wtile, rhs=wtile, start=True, stop=True)

        # ---- constants
        identb = sb.tile([P, P], dtb, tag="identb", name="identb")
        nc.vector.tensor_copy(identb, identg)
        negIb = sb.tile([P, P], dtb, tag="negIb", name="negIb")
        nc.vector.tensor_scalar_mul(negIb, identg, -1.0)
        negI2b = sb.tile([P, P], dtb, tag="negI2b", name="negI2b")
        nc.vector.tensor_scalar_mul(negI2b, identg, -2.0)
        ones11 = sb.tile([NF, 1], dt, tag="ones11", name="ones11")
        nc.vector.memset(ones11, 1.0)


        dinv_row = {}
        d2row = {}
        ah2 = {}
        t2 = {}
        t4d = {}
        for g in range(GPC):
            ah2[g] = sb.tile([P, NCH, N], dtb, tag=f"ah{g}", name=f"ah{g}")
            t2[g] = sb.tile([P, NCH, N], dtb, tag=f"t2{g}", name=f"t2{g}")
            t4d[g] = sb.tile([P, NCH, N], dtb, tag=f"t4{g}", name=f"t4{g}")

        with tc.tile_pool(name="psb", bufs=3, space="PSUM") as psb, \
             tc.tile_pool(name="psv", bufs=2, space="PSUM") as psv, \
             tc.tile_pool(name="psx", bufs=1, space="PSUM") as psx:

            def prep_graph(g):
                # adjacency entries arrive as 2.0 (host-folded factor), so the
                # reduce gives 2*deg.  All elementwise work happens in column
                # layout on 128 DVE lanes ([1,N] single-partition DVE ops are
                # ~20x slower — a [1,512] reciprocal measured 3.3us).
                degc = sb.tile([P, NCH], dt, tag=f"degc{g}", name=f"degc{g}")
                for kk in range(NCH):
                    nc.vector.tensor_reduce(degc[:, kk:kk + 1], adj0[g][:, kk, :],
                                            axis=mybir.AxisListType.X, op=Alu.add)
                dmaxc = sb.tile([P, NCH], dt, tag=f"dmaxc{g}", name=f"dmaxc{g}")
                nc.vector.tensor_scalar(dmaxc, degc, 0.5, 1.0, Alu.mult, Alu.max)
                srootc = sb.tile([P, NCH], dt, tag=f"srootc{g}", name=f"srootc{g}")
                nc.scalar.sqrt(srootc, dmaxc)
                dinvc = sb.tile([P, NCH], dt, tag=f"dinvc{g}", name=f"dinvc{g}")
                nc.vector.reciprocal(dinvc, srootc)
                pscr = psv.tile([S, N], dt, tag="cr", name="cr")[:1, :]
                for kk in range(NCH):
                    nc.tensor.transpose(pscr[:, kk * P:(kk + 1) * P],
                                        dinvc[:, kk:kk + 1], identg)
                dinv_row[g] = sb.tile([1, N], dtb, tag=f"dinv{g}", name=f"dinv{g}")
                nc.vector.tensor_copy(dinv_row[g], pscr)
                # ah2 = 2*Ahat: bf16 rank-1 outer product, masked by adj (=2)
                for kk in range(NCH):
                    dps = psb.tile([P, N], dt, tag="big", name="big")
                    nc.tensor.matmul(dps, lhsT=dinv_row[g][:, kk * P:(kk + 1) * P],
                                     rhs=dinv_row[g], start=True, stop=True)
                    nc.vector.tensor_tensor(ah2[g][:, kk, :],
                                            adj0[g][:, kk, :], dps, Alu.mult)

            # squarings: T2 = (ah2@ah2)/2 - I ; t4d = 4*T2@T2 - 2I (all bf16)
            def square_into(src_m, dst_map, g, scale, dI):
                for m in range(NCH):
                    ps = psb.tile([P, N], dt, tag="big", name="big")
                    for kk in range(NCH):
                        nc.tensor.matmul(ps, lhsT=src_m[g][:, kk, m * P:(m + 1) * P],
                                         rhs=src_m[g][:, kk, :],
                                         start=(kk == 0), stop=(kk == NCH - 1))
                    t = dst_map[g]
                    h = N // 2
                    nc.vector.tensor_scalar_mul(t[:, m, :h], ps[:, :h], scale)
                    nc.scalar.mul(t[:, m, h:], ps[:, h:], scale)
                    nc.vector.tensor_add(t[:, m, m * P:(m + 1) * P],
                                         t[:, m, m * P:(m + 1) * P], dI)

            # prep(1) is emitted after T2(0) so its slow DVE reduce chain
            # overlaps the first squaring instead of blocking its start
            prep_graph(0)
            square_into(ah2, t2, 0, 0.5, negIb)
            prep_graph(1)
            square_into(ah2, t2, 1, 0.5, negIb)
            square_into(t2, t4d, 0, 4.0, negI2b)
            square_into(t2, t4d, 1, 4.0, negI2b)

            # ---- gamma tiles (unnormalized, bf16), nnr = -(1/||C||)/N, and
            # c1 = (1 - rnorm*gamma00)/N folded from the u-row.  Emitted after
            # the squarings: only needed by the w stage.
            gam = {}
            nnr = sb.tile([1, 1], dt, tag="nnr", name="nnr")
            c1s = sb.tile([1, 1], dt, tag="c1s", name="c1s")
            csq = sb.tile([NF, 1], dt, tag="csq", name="csq")
            nc.vector.tensor_mul(csq, cvec, cvec)
            ps1 = psv.tile([S, N], dt, tag="cr", name="cr")[:1, :1]
            nc.tensor.matmul(ps1, lhsT=csq, rhs=ones11, start=True, stop=True)
            snorm = sb.tile([1, 1], dt, tag="snorm", name="snorm")
            nc.scalar.sqrt(snorm, ps1)
            rnorm = sb.tile([1, 1], dt, tag="rnorm", name="rnorm")
            nc.vector.reciprocal(rnorm, snorm)
            nc.vector.tensor_scalar_mul(nnr, rnorm, -1.0 / N)
            # slices of the 12 flat coefficients: [c00 | c01 | c02 c03 | q1 | q2]
            gam00f = sb.tile([1, 1], dt, tag="gam00f", name="gam00f")
            for key, lo, hi in (("c00", 0, 1), ("c01", 1, 2), ("c023", 2, 4),
                                ("q1", 4, 8), ("q2", 8, 12)):
                psq = psv.tile([S, N], dt, tag="cr", name="cr")[:hi - lo, :1]
                nc.tensor.matmul(psq, lhsT=gmat[:, lo:hi], rhs=cvec,
                                 start=True, stop=True)
                if key == "c00":
                    nc.vector.tensor_copy(gam00f, psq)
                else:
                    gam[key] = sb.tile([hi - lo, 1], dtb, tag=f"gam_{key}",
                                       name=f"gam_{key}")
                    nc.vector.tensor_copy(gam[key], psq)
            tt = sb.tile([1, 1], dt, tag="tt", name="tt")
            nc.vector.tensor_mul(tt, rnorm, gam00f)
            nc.vector.tensor_scalar(c1s, tt, -1.0 / N, 1.0 / N, Alu.mult, Alu.add)

            # ---- baby vectors + giant chain (bf16 storage, fp32 PSUM)
            gcol = {}
            z1col = {}
            for g in range(GPC):
                gcol[g] = sb.tile([P, NCH, S], dtb, tag=f"gc{g}", name=f"gc{g}")
                nc.gpsimd.memset(gcol[g][:, :, 0:1], 1.0)
                z1col[g] = sb.tile([P, NCH, S], dtb, tag=f"zc{g}", name=f"zc{g}")

            # PSUM evictions alternate engines per graph so the two
            # graphs' chains drain in parallel (each PSUM-touching DVE/ACT op
            # costs ~150-700ns serially on its engine)
            def ev_copy(g, out, in_):
                nc.vector.tensor_copy(out, in_)

            r1 = {}
            r23 = {}
            z1row = {}
            z2row = {}
            # babies pass 1: g1 = (ah2 @ 1)/2
            onesb = sb.tile([P, 1], dtb, tag="onesb", name="onesb")
            nc.vector.memset(onesb, 1.0)
            for g in range(GPC):
                r1[g] = sb.tile([1, N], dtb, tag=f"r1{g}", name=f"r1{g}")
                ps = psv.tile([S, N], dt, tag="cr", name="cr")[:1, :]
                for kk in range(NCH):
                    nc.tensor.matmul(ps, lhsT=onesb, rhs=ah2[g][:, kk, :],
                                     start=(kk == 0), stop=(kk == NCH - 1))
                ev_copy(g, r1[g], ps)
            # transpose g1 row -> gcol col 1 (stride-2 slots keep PSUM 4B-aligned)
            for g in range(GPC):
                pst = psv.tile([P, NCH * S], dtb, tag="tp", name="tp")[:, :NCH * 2]
                for kk in range(NCH):
                    nc.tensor.transpose(pst[:, kk * 2:kk * 2 + 1],
                                        r1[g][:, kk * P:(kk + 1) * P], identb[:1, :1])
                ev_copy(g, gcol[g][:, :, 1:2],
                        pst.rearrange("p (c two) -> p c two", two=2)[:, :, 0:1])
            # babies pass 2: stream t2 with stationary [u, g1]:
            #   row0 = T2@u = g2 ; row1 = T2@T1@u = h3 (raw; gamma absorbs)
            for g in range(GPC):
                r23[g] = sb.tile([2, N], dtb, tag=f"r23{g}", name=f"r23{g}")
                ps = psv.tile([S, N], dt, tag="cr", name="cr")[:2, :]
                for kk in range(NCH):
                    nc.tensor.matmul(ps, lhsT=gcol[g][:, kk, 0:2], rhs=t2[g][:, kk, :],
                                     start=(kk == 0), stop=(kk == NCH - 1))
                ev_copy(g, r23[g], ps)
            # transpose g2,g3 rows -> gcol cols 2,3
            for g in range(GPC):
                pst = psv.tile([P, NCH * S], dtb, tag="tp", name="tp")[:, :NCH * 2]
                for kk in range(NCH):
                    nc.tensor.transpose(pst[:, kk * 2:(kk + 1) * 2],
                                        r23[g][:, kk * P:(kk + 1) * P], identb[:2, :2])
                ev_copy(g, gcol[g][:, :, 2:4],
                        pst.rearrange("p (c s) -> p c s", s=2))

            # chain step 1: Z1 = T4 @ G   (= t4d@G / 2)
            for g in range(GPC):
                z1row[g] = sb.tile([S, N], dtb, tag=f"z1r{g}", name=f"z1r{g}")
                ps = psv.tile([S, N], dt, tag="cr", name="cr")
                for kk in range(NCH):
                    nc.tensor.matmul(ps, lhsT=gcol[g][:, kk, :], rhs=t4d[g][:, kk, :],
                                     start=(kk == 0), stop=(kk == NCH - 1))
                ev_copy(g, z1row[g], ps)
            for g in range(GPC):
                pst = psv.tile([P, NCH * S], dtb, tag="tp", name="tp")
                for kk in range(NCH):
                    nc.tensor.transpose(pst[:, kk * S:(kk + 1) * S],
                                        z1row[g][:, kk * P:(kk + 1) * P], identb[:S, :S])
                ev_copy(g, z1col[g].rearrange("p c s -> p (c s)"), pst)
            # ---- w accumulation starts EARLY: the first 3 of 4 matmuls only
            # need r1/r23/z1row, so they run before the z2 chain pass instead
            # of serializing after it.  The held wps accumulators occupy both
            # "cr" buffers, so the z2 passes get their own 1-buffer pool
            # (8th PSUM bank).
            wps = {}
            for g in range(GPC):
                wps[g] = psv.tile([S, N], dt, tag="cr", name="cr")[:1, :]
                nc.tensor.matmul(wps[g], lhsT=gam["c01"], rhs=r1[g],
                                 start=True, stop=False, skip_group_check=True)
                nc.tensor.matmul(wps[g], lhsT=gam["c023"], rhs=r23[g],
                                 start=False, stop=False, skip_group_check=True)
                nc.tensor.matmul(wps[g], lhsT=gam["q1"], rhs=z1row[g],
                                 start=False, stop=False, skip_group_check=True)
            # chain step 2: Z2' = t4d@Z1 (raw; gamma absorbs the -G term)
            for g in range(GPC):
                z2row[g] = sb.tile([S, N], dtb, tag=f"z2r{g}", name=f"z2r{g}")
                ps = psx.tile([S, N], dt, tag="cz", name="cz")
                for kk in range(NCH):
                    nc.tensor.matmul(ps, lhsT=z1col[g][:, kk, :], rhs=t4d[g][:, kk, :],
                                     start=(kk == 0), stop=(kk == NCH - 1))
                ev_copy(g, z2row[g], ps)

            #     w = c01*g1 + c023^T r23 + q1^T Z1 + q2^T Z2'
            #     v = c1s + nnr*w ; emb = v^T X (bf16)
            vrow = {}
            vcol = {}
            for g in range(GPC):
                nc.tensor.matmul(wps[g], lhsT=gam["q2"], rhs=z2row[g],
                                 start=False, stop=True, skip_group_check=True)
                vrow[g] = sb.tile([1, N], dtb, tag=f"vrow{g}", name=f"vrow{g}")
                nc.scalar.activation(vrow[g], wps[g],
                                     mybir.ActivationFunctionType.Identity,
                                     bias=c1s[:, 0:1], scale=nnr[:, 0:1])
            for g in range(GPC):
                pst = psv.tile([P, NCH * S], dtb, tag="tp", name="tp")[:, :NCH * 2]
                for kk in range(NCH):
                    nc.tensor.transpose(pst[:, kk * 2:kk * 2 + 1],
                                        vrow[g][:, kk * P:(kk + 1) * P], identb[:1, :1])
                vcol[g] = sb.tile([P, NCH], dtb, tag=f"vc{g}", name=f"vc{g}")
                ev_copy(g, vcol[g],
                        pst.rearrange("p (c two) -> p c two", two=2)[:, :, 0])
            for g in range(GPC):
                pse = psv.tile([S, N], dt, tag="cr", name="cr")[:1, :F]
                for kk in range(NCH):
                    nc.tensor.matmul(pse, lhsT=vcol[g][:, kk:kk + 1],
                                     rhs=xs[g][:, kk, :],
                                     start=(kk == 0), stop=(kk == NCH - 1))
                erow = sb.tile([1, F], dt, tag=f"erow{g}", name=f"erow{g}")
                ev_copy(g, erow, pse)
                nc.scalar.dma_start(emb_d[g:g + 1, :], erow)


# ---------------------------------------------------------------------------
# host: final loss from embeddings (float64; same bookkeeping the reference
# does on the host with numpy: class index construction / product combos)
def final_loss(emb, C, y):
    from itertools import product as _product
    e = emb.astype(np.float64)
    sq = (e * e).sum(1)
    D2 = sq[:, None] + sq[None, :] - 2 * e @ e.T
    D = np.sqrt(np.maximum(D2, 0.0))
    np.fill_diagonal(D, 0.0)
    y = np.asarray(y)
    class_idx = [np.nonzero(y == i)[0] for i in range(K)]
    neg = np.array(list(_product(*class_idx)))
    h1 = -sum(D[np.ix_(cb, cb)].mean() for cb in neg)
    h2 = sum(D[np.ix_(ci, ci)].mean() for ci in class_idx)
    beta = neg.shape[0] / K
    C64 = np.asarray(C, np.float64)
    dims = np.sqrt(float(C64.shape[0]))
    l1 = np.abs(C64).sum(0)
    l2 = np.sqrt((C64 * C64).sum(0))
    sparsity = np.mean((dims - l1 / l2) / (dims - 1))
    return sparsity + h2 + h1 / beta


# ---------------------------------------------------------------------------
_COMPILED = {}


def _get_nc():
    if "nc" in _COMPILED:
        return _COMPILED["nc"]
    import concourse.mybir as mybir
    import concourse.tile as tile
    from concourse import bacc

    dt = mybir.dt.float32
    nc = bacc.Bacc("TRN2", target_bir_lowering=False, debug=False)
    adj_d = nc.dram_tensor("adj", [GPC, P, NCH * N], mybir.dt.bfloat16,
                           kind="ExternalInput").ap()
    x_d = nc.dram_tensor("x", [GPC, P, NCH * F], mybir.dt.bfloat16,
                         kind="ExternalInput").ap()
    c_d = nc.dram_tensor("cvec", [NF, 1], dt, kind="ExternalInput").ap()
    g_d = nc.dram_tensor("gmat", [NF, NG], dt, kind="ExternalInput").ap()
    emb_d = nc.dram_tensor("emb", [GPC, F], dt, kind="ExternalOutput").ap()

    with tile.TileContext(nc) as tc:
        build_device_kernel(tc, emb_d, (adj_d, x_d, c_d, g_d))
    nc.compile()

    _COMPILED["nc"] = nc
    return nc


def kernel(adj, x, C, y):
    global LAST_EXEC_NS, LAST_RESULTS
    from concourse.bass_utils import run_bass_kernel_spmd
    import ml_dtypes

    # adjacency ships as bf16 with entries 2.0 (exact): folds the 2*Ahat
    # factor into the mask multiply; x tolerates bf16 (the emb mean averages
    # the rounding noise far below the accuracy gate).  Both are pre-arranged
    # partition-major ([g, p, chunk*inner]) so device DMAs are contiguous.
    adj = (np.asarray(adj, np.float32) * 2.0).astype(ml_dtypes.bfloat16)
    adj = np.ascontiguousarray(
        adj.reshape(G, NCH, P, N).transpose(0, 2, 1, 3).reshape(G, P, NCH * N))
    x = np.asarray(x, np.float32).astype(ml_dtypes.bfloat16)
    x = np.ascontiguousarray(
        x.reshape(G, NCH, P, F).transpose(0, 2, 1, 3).reshape(G, P, NCH * F))
    C = np.ascontiguousarray(np.asarray(C, np.float32))

    nc = _get_nc()
    in_maps = []
    for c in range(NCORES):
        in_maps.append({
            "adj": adj[c * GPC:(c + 1) * GPC],
            "x": x[c * GPC:(c + 1) * GPC],
            "cvec": C,
            "gmat": GAMMA_MAT,
        })
    import time as _time
    for attempt in range(3):
        try:
            res = run_bass_kernel_spmd(nc, in_maps, core_ids=list(range(NCORES)), trace=TRACE)
            break
        except Exception:
            # transient device errors (e.g. NRT_EXEC_UNIT_UNRECOVERABLE from a
            # previously killed process) clear after a moment
            if attempt == 2:
                raise
            _time.sleep(2.0)
    LAST_EXEC_NS = res.exec_time_ns
    LAST_RESULTS = res
    emb = np.concatenate([res.results[c]["emb"] for c in range(NCORES)], axis=0)
    loss = final_loss(emb, C, y)
    return np.float32(loss)


# revision 73
# speedup vs baseline: 1.0874x; 1.0696x over previous
"""Trainium2 Bass kernel for nn_DictNet_44547400794580.

Math: the loss only needs each graph's embedding
    emb_g = (1/N) * (1 - w_g)^T X_g,   w_g = sum_f c_f * (40(L_g - b_f I)^4 + I)^(-2) @ 1
where L_g = I - Ahat_g (sym-normalized Laplacian) and c = C/||C||_2.
All 11 filters are fixed rational functions of Ahat_g (spectrum in [-1,1]); the
combined filter is approximated by ONE degree-11 Chebyshev polynomial (final
loss rel err ~3e-4 host-side vs the 2e-2 gate) evaluated with a
baby-step/giant-step scheme in the product basis T_r(x)*T_q(T_4(x)), r<4, q<3:
  - 2 matrix squarings build T_2, T_4 of Ahat
  - 3 baby vectors via 2 streamed passes (t2 pass carries 2 stationary cols)
  - 2 giant chain steps in T_4 over the 4-column baby block
  - w accumulated by 4 tiny-K matmuls over the row stages
Perf structure:
  - adjacency ships as bf16 with entries 2.0 (exact for a 0/1 matrix; folds
    the 2*Ahat factor), x as bf16; all device matrices are bf16 (PE streams
    bf16/fp32r at the same 1 col/cycle; PSUM accumulates fp32; bf16 storage
    rounding adds ~2e-4 to a 2e-2 loss gate)
  - host pre-arranges adj/x partition-major so every DMA transfer is
    >=2KB-contiguous per partition (small strided segments crawl on the DGEs)
  - bulk DMA on the scalar HW DGE + gpsimd SW DGE queues only (the sync HW
    queue measured ~14GB/s; the scalar queue starves while the scalar engine
    computes, so x rides gpsimd, whose engine idles during the main phase)
  - a ~130-matmul PE warm-up spin on a dedicated tile releases the HAM clock
    gate (1.2 -> 2.4 GHz after ~3.4us sustained) and bridges the DMA/degree
    prologue so the squarings run at full clock
  - every PSUM eviction is a raw same-dtype copy (the DVE fast path; scaled
    or dtype-converting few-partition ops cost ~680ns each) — all stage
    scales are absorbed into the host-side gamma coefficients
  - C-normalization, 1/N, and the constant u-row of the w accumulation fold
    into one scalar-engine [1,N] affine at the end
Sharding: data-parallel over graphs, 2 graphs per core on 8 cores.  The host
gathers the (tiny) [16,256] embeddings and does the final cdist/sparsity
reduction in float64 — the same index bookkeeping the reference itself
performs on the host with numpy.
"""
import sys
if '/opt/trn_rl_repo' not in sys.path:
    sys.path.insert(0, '/opt/trn_rl_repo')

import numpy as np

# ---------------------------------------------------------------------------
# problem constants (hardcoded per contract)
G, N, F, K, NF = 16, 512, 256, 4, 11
NCORES = 8
GPC = G // NCORES          # graphs per core
P = 128
NCH = N // P               # 512 = 4 partition chunks
DEG = 7                    # Chebyshev degree (host rel err ~7e-4 at D=7)
S = 4                      # baby steps
MQ = DEG // S + 1          # giant columns q = 0..2
NG = S * MQ                # 12 product-basis coefficients
NWARM = 130                 # PE warm-up matmuls (~107ns each at cold clock)


# ---------------------------------------------------------------------------
# host-side fixed constants: Chebyshev coefficients of the 11 filters in the
# product basis, as a [NF, NG] matrix (pure math, no input data).
def _build_gamma_mat():
    bs = np.linspace(0.0, 2.0, NF)

    def psi(a, b):
        return (40.0 * (1.0 - a - b) ** 4 + 1.0) ** (-2)

    k = np.arange(DEG + 1)
    xk = np.cos(np.pi * (k + 0.5) / (DEG + 1))
    Mx = np.cos(k[:, None] * np.pi * (k[None, :] + 0.5) / (DEG + 1))

    gm = np.zeros((NF, NG))
    for fi, b in enumerate(bs):
        c = 2.0 / (DEG + 1) * (Mx @ psi(xk, b))
        c[0] *= 0.5
        beta = c.copy()
        gamma = np.zeros((S, MQ))
        for kk in range(DEG, S - 1, -1):
            q, r = divmod(kk, S)
            if r == 0:
                gamma[0, q] = beta[kk]
            else:
                gamma[r, q] = 2.0 * beta[kk]
                beta[S * q - r] -= beta[kk]
        for r in range(S):
            gamma[r, 0] += beta[r]
        # the device stores every stage RAW (pure PSUM-copy evictions, which
        # hit the DVE fast path; scaled [few-partition] tensor_scalar ops
        # cost ~680ns each).  Stored bases:
        #   col1 = 2*T1u (ah2@u), col2 = T2u, col3 = T3u+T1u (t2@col1)
        #   Z1'' = t4d@G (= 2*T1(W)G), Z2'' = t4d@Z1'' (= 2*(T2(W)+I)G)
        gamma[1, :] = (gamma[1, :] - gamma[3, :]) / 2.0
        if MQ > 2:
            gamma[:, 0] -= gamma[:, 2]
            gamma[:, 2] /= 2.0
        gamma[:, 1] /= 2.0
        # flatten q-major: index q*S + r
        gm[fi] = gamma.T.reshape(-1)
    return gm.astype(np.float32)


GAMMA_MAT = _build_gamma_mat()          # [11, 12]

TRACE = False
LAST_EXEC_NS = None
LAST_RESULTS = None


# ---------------------------------------------------------------------------
# device kernel (one core: GPC graphs)
def build_device_kernel(tc, outs, ins):
    import concourse.mybir as mybir
    from concourse.masks import make_identity
    from contextlib import ExitStack

    nc = tc.nc
    dt = mybir.dt.float32
    dtr = mybir.dt.float32r
    dtb = mybir.dt.bfloat16
    Alu = mybir.AluOpType

    def mmr(out, lhsT, rhs, **kw):
        nc.tensor.matmul(out, lhsT=lhsT.bitcast(dtr), rhs=rhs.bitcast(dtr), **kw)

    adj_d, x_d, c_d, g_d = ins
    emb_d = outs

    with ExitStack() as ctx:
        ctx.enter_context(nc.allow_low_precision(
            reason="bf16 matrices are intentional: PSUM accumulates fp32, "
                   "bf16 storage rounding adds ~1e-4 to a 2e-2 gate"))
        sb = ctx.enter_context(tc.tile_pool(name="sb", bufs=1))

        adj0 = {}
        xs = {}
        for g in range(GPC):
            adj0[g] = sb.tile([P, NCH, N], dtb, tag=f"adj0_{g}", name=f"adj0_{g}")
            xs[g] = sb.tile([P, NCH, F], dtb, tag=f"xin_{g}", name=f"xin_{g}")

        # warm-up source: first vector-engine op, no other dependencies
        wtile = sb.tile([P, P], dtb, tag="wtile", name="wtile")
        nc.vector.memset(wtile, 0.5)

        # the host pre-arranges adj and x into partition-major layout, so
        # every transfer is >=2KB-contiguous per partition (small strided
        # segments crawl on both the HW and SW DGE queues)
        def adj_half(g, h):
            return adj_d[g][:, h * 2 * N:(h + 1) * 2 * N].rearrange(
                "p (c n) -> p c n", n=N)

        # identity first on gpsimd (the DVE constant chain hangs off it),
        # then that queue's DMA issues
        identg = sb.tile([P, P], dt, tag="identg", name="identg")
        make_identity(nc, identg)

        cvec = sb.tile([NF, 1], dt, tag="cvec", name="cvec")
        gmat = sb.tile([NF, NG], dt, tag="gmat", name="gmat")
        # x rides the gpsimd queue only: the scalar HW queue starves while
        # the scalar engine runs evictions (port contention), gpsimd idles
        nc.scalar.dma_start(adj0[0][:, 0:2, :], adj_half(0, 0))
        nc.gpsimd.dma_start(adj0[0][:, 2:4, :], adj_half(0, 1))
        nc.scalar.dma_start(adj0[1][:, 0:2, :], adj_half(1, 0))
        nc.gpsimd.dma_start(adj0[1][:, 2:4, :], adj_half(1, 1))
        nc.gpsimd.dma_start(xs[1], x_d[1].rearrange("p (c f) -> p c f", f=F))
        nc.gpsimd.dma_start(xs[0], x_d[0].rearrange("p (c f) -> p c f", f=F))
        # tiny constants ride the (slow but sufficient) sync queue so the
        # scalar engine queue is free for its sqrt after just 2 DMA issues
        nc.sync.dma_start(cvec, c_d)
        nc.sync.dma_start(gmat, g_d)

        # ---- PE warm-up spin (HAM clock gate releases after ~3.4us busy)
        with tc.tile_pool(name="pwm", bufs=1, space="PSUM") as pwm:
            ps_warm = pwm.tile([P, P], dt, tag="warm", name="warm")
            for _ in range(NWARM):
                nc.tensor.matmul(ps_warm, lhsT=wtile, rhs=wtile, start=True, stop=True)

        # ---- constants
        identb = sb.tile([P, P], dtb, tag="identb", name="identb")
        nc.vector.tensor_copy(identb, identg)
        negIb = sb.tile([P, P], dtb, tag="negIb", name="negIb")
        nc.vector.tensor_scalar_mul(negIb, identg, -1.0)
        negI2b = sb.tile([P, P], dtb, tag="negI2b", name="negI2b")
        nc.vector.tensor_scalar_mul(negI2b, identg, -2.0)
        ones11 = sb.tile([NF, 1], dt, tag="ones11", name="ones11")
        nc.vector.memset(ones11, 1.0)


        dinv_row = {}
        d2row = {}
        ah2 = {}
        t2 = {}
        t4d = {}
        for g in range(GPC):
            ah2[g] = sb.tile([P, NCH, N], dtb, tag=f"ah{g}", name=f"ah{g}")
            t2[g] = sb.tile([P, NCH, N], dtb, tag=f"t2{g}", name=f"t2{g}")
            t4d[g] = sb.tile([P, NCH, N], dtb, tag=f"t4{g}", name=f"t4{g}")

        with tc.tile_pool(name="psb", bufs=3, space="PSUM") as psb, \
             tc.tile_pool(name="psv", bufs=2, space="PSUM") as psv:

            def prep_graph(g):
                # adjacency entries arrive as 2.0 (host-folded factor), so the
                # reduce gives 2*deg.  All elementwise work happens in column
                # layout on 128 DVE lanes ([1,N] single-partition DVE ops are
                # ~20x slower — a [1,512] reciprocal measured 3.3us).
                degc = sb.tile([P, NCH], dt, tag=f"degc{g}", name=f"degc{g}")
                for kk in range(NCH):
                    nc.vector.tensor_reduce(degc[:, kk:kk + 1], adj0[g][:, kk, :],
                                            axis=mybir.AxisListType.X, op=Alu.add)
                dmaxc = sb.tile([P, NCH], dt, tag=f"dmaxc{g}", name=f"dmaxc{g}")
                nc.vector.tensor_scalar(dmaxc, degc, 0.5, 1.0, Alu.mult, Alu.max)
                srootc = sb.tile([P, NCH], dt, tag=f"srootc{g}", name=f"srootc{g}")
                nc.scalar.sqrt(srootc, dmaxc)
                dinvc = sb.tile([P, NCH], dt, tag=f"dinvc{g}", name=f"dinvc{g}")
                nc.vector.reciprocal(dinvc, srootc)
                pscr = psv.tile([S, N], dt, tag="cr", name="cr")[:1, :]
                for kk in range(NCH):
                    nc.tensor.transpose(pscr[:, kk * P:(kk + 1) * P],
                                        dinvc[:, kk:kk + 1], identg)
                dinv_row[g] = sb.tile([1, N], dtb, tag=f"dinv{g}", name=f"dinv{g}")
                nc.vector.tensor_copy(dinv_row[g], pscr)
                # ah2 = 2*Ahat: bf16 rank-1 outer product, masked by adj (=2)
                for kk in range(NCH):
                    dps = psb.tile([P, N], dt, tag="big", name="big")
                    nc.tensor.matmul(dps, lhsT=dinv_row[g][:, kk * P:(kk + 1) * P],
                                     rhs=dinv_row[g], start=True, stop=True)
                    nc.vector.tensor_tensor(ah2[g][:, kk, :],
                                            adj0[g][:, kk, :], dps, Alu.mult)

            # squarings: T2 = (ah2@ah2)/2 - I ; t4d = 4*T2@T2 - 2I (all bf16)
            def square_into(src_m, dst_map, g, scale, dI):
                for m in range(NCH):
                    ps = psb.tile([P, N], dt, tag="big", name="big")
                    for kk in range(NCH):
                        nc.tensor.matmul(ps, lhsT=src_m[g][:, kk, m * P:(m + 1) * P],
                                         rhs=src_m[g][:, kk, :],
                                         start=(kk == 0), stop=(kk == NCH - 1))
                    t = dst_map[g]
                    h = N // 2
                    nc.vector.tensor_scalar_mul(t[:, m, :h], ps[:, :h], scale)
                    nc.scalar.mul(t[:, m, h:], ps[:, h:], scale)
                    nc.vector.tensor_add(t[:, m, m * P:(m + 1) * P],
                                         t[:, m, m * P:(m + 1) * P], dI)

            # prep(1) is emitted after T2(0) so its slow DVE reduce chain
            # overlaps the first squaring instead of blocking its start
            prep_graph(0)
            square_into(ah2, t2, 0, 0.5, negIb)
            prep_graph(1)
            square_into(ah2, t2, 1, 0.5, negIb)
            square_into(t2, t4d, 0, 4.0, negI2b)
            square_into(t2, t4d, 1, 4.0, negI2b)

            # ---- gamma tiles (unnormalized, bf16), nnr = -(1/||C||)/N, and
            # c1 = (1 - rnorm*gamma00)/N folded from the u-row.  Emitted after
            # the squarings: only needed by the w stage.
            gam = {}
            nnr = sb.tile([1, 1], dt, tag="nnr", name="nnr")
            c1s = sb.tile([1, 1], dt, tag="c1s", name="c1s")
            csq = sb.tile([NF, 1], dt, tag="csq", name="csq")
            nc.vector.tensor_mul(csq, cvec, cvec)
            ps1 = psv.tile([S, N], dt, tag="cr", name="cr")[:1, :1]
            nc.tensor.matmul(ps1, lhsT=csq, rhs=ones11, start=True, stop=True)
            snorm = sb.tile([1, 1], dt, tag="snorm", name="snorm")
            nc.scalar.sqrt(snorm, ps1)
            rnorm = sb.tile([1, 1], dt, tag="rnorm", name="rnorm")
            nc.vector.reciprocal(rnorm, snorm)
            nc.vector.tensor_scalar_mul(nnr, rnorm, -1.0 / N)
            # slices of the 12 flat coefficients: [c00 | c01 | c02 c03 | q1 | q2]
            gam00f = sb.tile([1, 1], dt, tag="gam00f", name="gam00f")
            for key, lo, hi in (("c00", 0, 1), ("c01", 1, 2), ("c023", 2, 4),
                                ("q1", 4, 8)):
                psq = psv.tile([S, N], dt, tag="cr", name="cr")[:hi - lo, :1]
                nc.tensor.matmul(psq, lhsT=gmat[:, lo:hi], rhs=cvec,
                                 start=True, stop=True)
                if key == "c00":
                    nc.vector.tensor_copy(gam00f, psq)
                else:
                    gam[key] = sb.tile([hi - lo, 1], dtb, tag=f"gam_{key}",
                                       name=f"gam_{key}")
                    nc.vector.tensor_copy(gam[key], psq)
            tt = sb.tile([1, 1], dt, tag="tt", name="tt")
            nc.vector.tensor_mul(tt, rnorm, gam00f)
            nc.vector.tensor_scalar(c1s, tt, -1.0 / N, 1.0 / N, Alu.mult, Alu.add)

            # ---- baby vectors + giant chain (bf16 storage, fp32 PSUM)
            gcol = {}
            for g in range(GPC):
                gcol[g] = sb.tile([P, NCH, S], dtb, tag=f"gc{g}", name=f"gc{g}")
                nc.gpsimd.memset(gcol[g][:, :, 0:1], 1.0)

            # PSUM evictions alternate engines per graph so the two
            # graphs' chains drain in parallel (each PSUM-touching DVE/ACT op
            # costs ~150-700ns serially on its engine)
            def ev_copy(g, out, in_):
                nc.vector.tensor_copy(out, in_)

            r1 = {}
            r23 = {}
            z1row = {}
            z2row = {}
            # babies pass 1: g1 = (ah2 @ 1)/2
            onesb = sb.tile([P, 1], dtb, tag="onesb", name="onesb")
            nc.vector.memset(onesb, 1.0)
            for g in range(GPC):
                r1[g] = sb.tile([1, N], dtb, tag=f"r1{g}", name=f"r1{g}")
                ps = psv.tile([S, N], dt, tag="cr", name="cr")[:1, :]
                for kk in range(NCH):
                    nc.tensor.matmul(ps, lhsT=onesb, rhs=ah2[g][:, kk, :],
                                     start=(kk == 0), stop=(kk == NCH - 1))
                ev_copy(g, r1[g], ps)
            # transpose g1 row -> gcol col 1 (stride-2 slots keep PSUM 4B-aligned)
            for g in range(GPC):
                pst = psv.tile([P, NCH * S], dtb, tag="tp", name="tp")[:, :NCH * 2]
                for kk in range(NCH):
                    nc.tensor.transpose(pst[:, kk * 2:kk * 2 + 1],
                                        r1[g][:, kk * P:(kk + 1) * P], identb[:1, :1])
                ev_copy(g, gcol[g][:, :, 1:2],
                        pst.rearrange("p (c two) -> p c two", two=2)[:, :, 0:1])
            # babies pass 2: stream t2 with stationary [u, g1]:
            #   row0 = T2@u = g2 ; row1 = T2@T1@u = h3 (raw; gamma absorbs)
            for g in range(GPC):
                r23[g] = sb.tile([2, N], dtb, tag=f"r23{g}", name=f"r23{g}")
                ps = psv.tile([S, N], dt, tag="cr", name="cr")[:2, :]
                for kk in range(NCH):
                    nc.tensor.matmul(ps, lhsT=gcol[g][:, kk, 0:2], rhs=t2[g][:, kk, :],
                                     start=(kk == 0), stop=(kk == NCH - 1))
                ev_copy(g, r23[g], ps)
            # transpose g2,g3 rows -> gcol cols 2,3
            for g in range(GPC):
                pst = psv.tile([P, NCH * S], dtb, tag="tp", name="tp")[:, :NCH * 2]
                for kk in range(NCH):
                    nc.tensor.transpose(pst[:, kk * 2:(kk + 1) * 2],
                                        r23[g][:, kk * P:(kk + 1) * P], identb[:2, :2])
                ev_copy(g, gcol[g][:, :, 2:4],
                        pst.rearrange("p (c s) -> p c s", s=2))

            # chain step 1: Z1 = T4 @ G   (= t4d@G / 2)
            for g in range(GPC):
                z1row[g] = sb.tile([S, N], dtb, tag=f"z1r{g}", name=f"z1r{g}")
                ps = psv.tile([S, N], dt, tag="cr", name="cr")
                for kk in range(NCH):
                    nc.tensor.matmul(ps, lhsT=gcol[g][:, kk, :], rhs=t4d[g][:, kk, :],
                                     start=(kk == 0), stop=(kk == NCH - 1))
                ev_copy(g, z1row[g], ps)
            #     w = c01*g1 + c023^T r23 + q1^T Z1
            #     v = c1s + nnr*w ; emb = v^T X (bf16)
            vrow = {}
            vcol = {}
            for g in range(GPC):
                wps = psv.tile([S, N], dt, tag="cr", name="cr")[:1, :]
                nc.tensor.matmul(wps, lhsT=gam["c01"], rhs=r1[g],
                                 start=True, stop=False, skip_group_check=True)
                nc.tensor.matmul(wps, lhsT=gam["c023"], rhs=r23[g],
                                 start=False, stop=False, skip_group_check=True)
                nc.tensor.matmul(wps, lhsT=gam["q1"], rhs=z1row[g],
                                 start=False, stop=True, skip_group_check=True)
                vrow[g] = sb.tile([1, N], dtb, tag=f"vrow{g}", name=f"vrow{g}")
                nc.scalar.activation(vrow[g], wps,
                                     mybir.ActivationFunctionType.Identity,
                                     bias=c1s[:, 0:1], scale=nnr[:, 0:1])
            for g in range(GPC):
                pst = psv.tile([P, NCH * S], dtb, tag="tp", name="tp")[:, :NCH * 2]
                for kk in range(NCH):
                    nc.tensor.transpose(pst[:, kk * 2:kk * 2 + 1],
                                        vrow[g][:, kk * P:(kk + 1) * P], identb[:1, :1])
                vcol[g] = sb.tile([P, NCH], dtb, tag=f"vc{g}", name=f"vc{g}")
                ev_copy(g, vcol[g],
                        pst.rearrange("p (c two) -> p c two", two=2)[:, :, 0])
            for g in range(GPC):
                pse = psv.tile([S, N], dt, tag="cr", name="cr")[:1, :F]
                for kk in range(NCH):
                    nc.tensor.matmul(pse, lhsT=vcol[g][:, kk:kk + 1],
                                     rhs=xs[g][:, kk, :],
                                     start=(kk == 0), stop=(kk == NCH - 1))
                erow = sb.tile([1, F], dt, tag=f"erow{g}", name=f"erow{g}")
                ev_copy(g, erow, pse)
                nc.scalar.dma_start(emb_d[g:g + 1, :], erow)


# ---------------------------------------------------------------------------
# host: final loss from embeddings (float64; same bookkeeping the reference
# does on the host with numpy: class index construction / product combos)
def final_loss(emb, C, y):
    from itertools import product as _product
    e = emb.astype(np.float64)
    sq = (e * e).sum(1)
    D2 = sq[:, None] + sq[None, :] - 2 * e @ e.T
    D = np.sqrt(np.maximum(D2, 0.0))
    np.fill_diagonal(D, 0.0)
    y = np.asarray(y)
    class_idx = [np.nonzero(y == i)[0] for i in range(K)]
    neg = np.array(list(_product(*class_idx)))
    h1 = -sum(D[np.ix_(cb, cb)].mean() for cb in neg)
    h2 = sum(D[np.ix_(ci, ci)].mean() for ci in class_idx)
    beta = neg.shape[0] / K
    C64 = np.asarray(C, np.float64)
    dims = np.sqrt(float(C64.shape[0]))
    l1 = np.abs(C64).sum(0)
    l2 = np.sqrt((C64 * C64).sum(0))
    sparsity = np.mean((dims - l1 / l2) / (dims - 1))
    return sparsity + h2 + h1 / beta


# ---------------------------------------------------------------------------
_COMPILED = {}


def _get_nc():
    if "nc" in _COMPILED:
        return _COMPILED["nc"]
    import concourse.mybir as mybir
    import concourse.tile as tile
    from concourse import bacc

    dt = mybir.dt.float32
    nc = bacc.Bacc("TRN2", target_bir_lowering=False, debug=False)
    adj_d = nc.dram_tensor("adj", [GPC, P, NCH * N], mybir.dt.bfloat16,
                           kind="ExternalInput").ap()
    x_d = nc.dram_tensor("x", [GPC, P, NCH * F], mybir.dt.bfloat16,
                         kind="ExternalInput").ap()
    c_d = nc.dram_tensor("cvec", [NF, 1], dt, kind="ExternalInput").ap()
    g_d = nc.dram_tensor("gmat", [NF, NG], dt, kind="ExternalInput").ap()
    emb_d = nc.dram_tensor("emb", [GPC, F], dt, kind="ExternalOutput").ap()

    with tile.TileContext(nc) as tc:
        build_device_kernel(tc, emb_d, (adj_d, x_d, c_d, g_d))
    nc.compile()

    _COMPILED["nc"] = nc
    return nc


def kernel(adj, x, C, y):
    global LAST_EXEC_NS, LAST_RESULTS
    from concourse.bass_utils import run_bass_kernel_spmd
    import ml_dtypes

    # adjacency ships as bf16 with entries 2.0 (exact): folds the 2*Ahat
    # factor into the mask multiply; x tolerates bf16 (the emb mean averages
    # the rounding noise far below the accuracy gate).  Both are pre-arranged
    # partition-major ([g, p, chunk*inner]) so device DMAs are contiguous.
    adj = (np.asarray(adj, np.float32) * 2.0).astype(ml_dtypes.bfloat16)
    adj = np.ascontiguousarray(
        adj.reshape(G, NCH, P, N).transpose(0, 2, 1, 3).reshape(G, P, NCH * N))
    x = np.asarray(x, np.float32).astype(ml_dtypes.bfloat16)
    x = np.ascontiguousarray(
        x.reshape(G, NCH, P, F).transpose(0, 2, 1, 3).reshape(G, P, NCH * F))
    C = np.ascontiguousarray(np.asarray(C, np.float32))

    nc = _get_nc()
    in_maps = []
    for c in range(NCORES):
        in_maps.append({
            "adj": adj[c * GPC:(c + 1) * GPC],
            "x": x[c * GPC:(c + 1) * GPC],
            "cvec": C,
            "gmat": GAMMA_MAT,
        })
    import time as _time
    for attempt in range(3):
        try:
            res = run_bass_kernel_spmd(nc, in_maps, core_ids=list(range(NCORES)), trace=TRACE)
            break
        except Exception:
            # transient device errors (e.g. NRT_EXEC_UNIT_UNRECOVERABLE from a
            # previously killed process) clear after a moment
            if attempt == 2:
                raise
            _time.sleep(2.0)
    LAST_EXEC_NS = res.exec_time_ns
    LAST_RESULTS = res
    emb = np.concatenate([res.results[c]["emb"] for c in range(NCORES)], axis=0)
    loss = final_loss(emb, C, y)
    return np.float32(loss)


# revision 74
# speedup vs baseline: 1.1544x; 1.0617x over previous
"""Trainium2 Bass kernel for nn_DictNet_44547400794580.

Math: the loss only needs each graph's embedding
    emb_g = (1/N) * (1 - w_g)^T X_g,   w_g = sum_f c_f * (40(L_g - b_f I)^4 + I)^(-2) @ 1
where L_g = I - Ahat_g (sym-normalized Laplacian) and c = C/||C||_2.
All 11 filters are fixed rational functions of Ahat_g (spectrum in [-1,1]); the
combined filter is approximated by ONE degree-11 Chebyshev polynomial (final
loss rel err ~3e-4 host-side vs the 2e-2 gate) evaluated with a
baby-step/giant-step scheme in the product basis T_r(x)*T_q(T_4(x)), r<4, q<3:
  - 2 matrix squarings build T_2, T_4 of Ahat
  - 3 baby vectors via 2 streamed passes (t2 pass carries 2 stationary cols)
  - 2 giant chain steps in T_4 over the 4-column baby block
  - w accumulated by 4 tiny-K matmuls over the row stages
Perf structure:
  - adjacency ships as bf16 with entries 2.0 (exact for a 0/1 matrix; folds
    the 2*Ahat factor), x as bf16; all device matrices are bf16 (PE streams
    bf16/fp32r at the same 1 col/cycle; PSUM accumulates fp32; bf16 storage
    rounding adds ~2e-4 to a 2e-2 loss gate)
  - host pre-arranges adj/x partition-major so every DMA transfer is
    >=2KB-contiguous per partition (small strided segments crawl on the DGEs)
  - bulk DMA on the scalar HW DGE + gpsimd SW DGE queues only (the sync HW
    queue measured ~14GB/s; the scalar queue starves while the scalar engine
    computes, so x rides gpsimd, whose engine idles during the main phase)
  - a ~130-matmul PE warm-up spin on a dedicated tile releases the HAM clock
    gate (1.2 -> 2.4 GHz after ~3.4us sustained) and bridges the DMA/degree
    prologue so the squarings run at full clock
  - every PSUM eviction is a raw same-dtype copy (the DVE fast path; scaled
    or dtype-converting few-partition ops cost ~680ns each) — all stage
    scales are absorbed into the host-side gamma coefficients
  - C-normalization, 1/N, and the constant u-row of the w accumulation fold
    into one scalar-engine [1,N] affine at the end
Sharding: data-parallel over graphs, 2 graphs per core on 8 cores.  The host
gathers the (tiny) [16,256] embeddings and does the final cdist/sparsity
reduction in float64 — the same index bookkeeping the reference itself
performs on the host with numpy.
"""
import sys
if '/opt/trn_rl_repo' not in sys.path:
    sys.path.insert(0, '/opt/trn_rl_repo')

import numpy as np

# ---------------------------------------------------------------------------
# problem constants (hardcoded per contract)
G, N, F, K, NF = 16, 512, 256, 4, 11
NCORES = 8
GPC = G // NCORES          # graphs per core
P = 128
NCH = N // P               # 512 = 4 partition chunks
DEG = 7                    # Chebyshev degree (host rel err ~7e-4 at D=7)
S = 4                      # baby steps
MQ = DEG // S + 1          # giant columns q = 0..2
NG = S * MQ                # 12 product-basis coefficients
NWARM = 130                 # PE warm-up matmuls (~107ns each at cold clock)


# ---------------------------------------------------------------------------
# host-side fixed constants: Chebyshev coefficients of the 11 filters in the
# product basis, as a [NF, NG] matrix (pure math, no input data).
def _build_gamma_mat():
    bs = np.linspace(0.0, 2.0, NF)

    def psi(a, b):
        return (40.0 * (1.0 - a - b) ** 4 + 1.0) ** (-2)

    k = np.arange(DEG + 1)
    xk = np.cos(np.pi * (k + 0.5) / (DEG + 1))
    Mx = np.cos(k[:, None] * np.pi * (k[None, :] + 0.5) / (DEG + 1))

    gm = np.zeros((NF, NG))
    for fi, b in enumerate(bs):
        c = 2.0 / (DEG + 1) * (Mx @ psi(xk, b))
        c[0] *= 0.5
        beta = c.copy()
        gamma = np.zeros((S, MQ))
        for kk in range(DEG, S - 1, -1):
            q, r = divmod(kk, S)
            if r == 0:
                gamma[0, q] = beta[kk]
            else:
                gamma[r, q] = 2.0 * beta[kk]
                beta[S * q - r] -= beta[kk]
        for r in range(S):
            gamma[r, 0] += beta[r]
        # the device stores every stage RAW (pure PSUM-copy evictions, which
        # hit the DVE fast path; scaled [few-partition] tensor_scalar ops
        # cost ~680ns each).  Stored bases:
        #   col1 = 2*T1u (ah2@u), col2 = T2u, col3 = T3u+T1u (t2@col1)
        #   Z1'' = t4d@G (= 2*T1(W)G), Z2'' = t4d@Z1'' (= 2*(T2(W)+I)G)
        gamma[1, :] = (gamma[1, :] - gamma[3, :]) / 2.0
        # z stage stored raw as T2^2 G ; T4 G = 2*z_raw - G folds into q0
        gamma[:, 0] -= gamma[:, 1]
        gamma[:, 1] *= 2.0
        # flatten q-major: index q*S + r
        gm[fi] = gamma.T.reshape(-1)
    return gm.astype(np.float32)


GAMMA_MAT = _build_gamma_mat()          # [11, 12]

TRACE = False
LAST_EXEC_NS = None
LAST_RESULTS = None


# ---------------------------------------------------------------------------
# device kernel (one core: GPC graphs)
def build_device_kernel(tc, outs, ins):
    import concourse.mybir as mybir
    from concourse.masks import make_identity
    from contextlib import ExitStack

    nc = tc.nc
    dt = mybir.dt.float32
    dtr = mybir.dt.float32r
    dtb = mybir.dt.bfloat16
    Alu = mybir.AluOpType

    def mmr(out, lhsT, rhs, **kw):
        nc.tensor.matmul(out, lhsT=lhsT.bitcast(dtr), rhs=rhs.bitcast(dtr), **kw)

    adj_d, x_d, c_d, g_d = ins
    emb_d = outs

    with ExitStack() as ctx:
        ctx.enter_context(nc.allow_low_precision(
            reason="bf16 matrices are intentional: PSUM accumulates fp32, "
                   "bf16 storage rounding adds ~1e-4 to a 2e-2 gate"))
        sb = ctx.enter_context(tc.tile_pool(name="sb", bufs=1))

        adj0 = {}
        xs = {}
        for g in range(GPC):
            adj0[g] = sb.tile([P, NCH, N], dtb, tag=f"adj0_{g}", name=f"adj0_{g}")
            xs[g] = sb.tile([P, NCH, F], dtb, tag=f"xin_{g}", name=f"xin_{g}")

        # warm-up source: first vector-engine op, no other dependencies
        wtile = sb.tile([P, P], dtb, tag="wtile", name="wtile")
        nc.vector.memset(wtile, 0.5)

        # the host pre-arranges adj and x into partition-major layout, so
        # every transfer is >=2KB-contiguous per partition (small strided
        # segments crawl on both the HW and SW DGE queues)
        def adj_half(g, h):
            return adj_d[g][:, h * 2 * N:(h + 1) * 2 * N].rearrange(
                "p (c n) -> p c n", n=N)

        # identity first on gpsimd (the DVE constant chain hangs off it),
        # then that queue's DMA issues
        identg = sb.tile([P, P], dt, tag="identg", name="identg")
        make_identity(nc, identg)

        cvec = sb.tile([NF, 1], dt, tag="cvec", name="cvec")
        gmat = sb.tile([NF, NG], dt, tag="gmat", name="gmat")
        # x rides the gpsimd queue only: the scalar HW queue starves while
        # the scalar engine runs evictions (port contention), gpsimd idles
        nc.scalar.dma_start(adj0[0][:, 0:2, :], adj_half(0, 0))
        nc.gpsimd.dma_start(adj0[0][:, 2:4, :], adj_half(0, 1))
        nc.scalar.dma_start(adj0[1][:, 0:2, :], adj_half(1, 0))
        nc.gpsimd.dma_start(adj0[1][:, 2:4, :], adj_half(1, 1))
        nc.gpsimd.dma_start(xs[1], x_d[1].rearrange("p (c f) -> p c f", f=F))
        nc.gpsimd.dma_start(xs[0], x_d[0].rearrange("p (c f) -> p c f", f=F))
        # tiny constants ride the (slow but sufficient) sync queue so the
        # scalar engine queue is free for its sqrt after just 2 DMA issues
        nc.sync.dma_start(cvec, c_d)
        nc.sync.dma_start(gmat, g_d)

        # ---- PE warm-up spin (HAM clock gate releases after ~3.4us busy)
        with tc.tile_pool(name="pwm", bufs=1, space="PSUM") as pwm:
            ps_warm = pwm.tile([P, P], dt, tag="warm", name="warm")
            for _ in range(NWARM):
                nc.tensor.matmul(ps_warm, lhsT=wtile, rhs=wtile, start=True, stop=True)

        # ---- constants
        identb = sb.tile([P, P], dtb, tag="identb", name="identb")
        nc.vector.tensor_copy(identb, identg)
        negIb = sb.tile([P, P], dtb, tag="negIb", name="negIb")
        nc.vector.tensor_scalar_mul(negIb, identg, -1.0)
        ones11 = sb.tile([NF, 1], dt, tag="ones11", name="ones11")
        nc.vector.memset(ones11, 1.0)


        dinv_row = {}
        d2row = {}
        ah2 = {}
        t2 = {}
        t4d = {}
        for g in range(GPC):
            ah2[g] = sb.tile([P, NCH, N], dtb, tag=f"ah{g}", name=f"ah{g}")
            t2[g] = sb.tile([P, NCH, N], dtb, tag=f"t2{g}", name=f"t2{g}")

        with tc.tile_pool(name="psb", bufs=3, space="PSUM") as psb, \
             tc.tile_pool(name="psv", bufs=2, space="PSUM") as psv:

            def prep_graph(g):
                # adjacency entries arrive as 2.0 (host-folded factor), so the
                # reduce gives 2*deg.  All elementwise work happens in column
                # layout on 128 DVE lanes ([1,N] single-partition DVE ops are
                # ~20x slower — a [1,512] reciprocal measured 3.3us).
                degc = sb.tile([P, NCH], dt, tag=f"degc{g}", name=f"degc{g}")
                for kk in range(NCH):
                    nc.vector.tensor_reduce(degc[:, kk:kk + 1], adj0[g][:, kk, :],
                                            axis=mybir.AxisListType.X, op=Alu.add)
                dmaxc = sb.tile([P, NCH], dt, tag=f"dmaxc{g}", name=f"dmaxc{g}")
                nc.vector.tensor_scalar(dmaxc, degc, 0.5, 1.0, Alu.mult, Alu.max)
                srootc = sb.tile([P, NCH], dt, tag=f"srootc{g}", name=f"srootc{g}")
                nc.scalar.sqrt(srootc, dmaxc)
                dinvc = sb.tile([P, NCH], dt, tag=f"dinvc{g}", name=f"dinvc{g}")
                nc.vector.reciprocal(dinvc, srootc)
                pscr = psv.tile([S, N], dt, tag="cr", name="cr")[:1, :]
                for kk in range(NCH):
                    nc.tensor.transpose(pscr[:, kk * P:(kk + 1) * P],
                                        dinvc[:, kk:kk + 1], identg)
                dinv_row[g] = sb.tile([1, N], dtb, tag=f"dinv{g}", name=f"dinv{g}")
                nc.vector.tensor_copy(dinv_row[g], pscr)
                # ah2 = 2*Ahat: bf16 rank-1 outer product, masked by adj (=2)
                for kk in range(NCH):
                    dps = psb.tile([P, N], dt, tag="big", name="big")
                    nc.tensor.matmul(dps, lhsT=dinv_row[g][:, kk * P:(kk + 1) * P],
                                     rhs=dinv_row[g], start=True, stop=True)
                    nc.vector.tensor_tensor(ah2[g][:, kk, :],
                                            adj0[g][:, kk, :], dps, Alu.mult)

            # squarings: T2 = (ah2@ah2)/2 - I ; t4d = 4*T2@T2 - 2I (all bf16)
            def square_into(src_m, dst_map, g, scale, dI):
                for m in range(NCH):
                    ps = psb.tile([P, N], dt, tag="big", name="big")
                    for kk in range(NCH):
                        nc.tensor.matmul(ps, lhsT=src_m[g][:, kk, m * P:(m + 1) * P],
                                         rhs=src_m[g][:, kk, :],
                                         start=(kk == 0), stop=(kk == NCH - 1))
                    t = dst_map[g]
                    h = N // 2
                    nc.vector.tensor_scalar_mul(t[:, m, :h], ps[:, :h], scale)
                    nc.scalar.mul(t[:, m, h:], ps[:, h:], scale)
                    nc.vector.tensor_add(t[:, m, m * P:(m + 1) * P],
                                         t[:, m, m * P:(m + 1) * P], dI)

            # prep(1) is emitted after T2(0) so its slow DVE reduce chain
            # overlaps the first squaring instead of blocking its start
            prep_graph(0)
            square_into(ah2, t2, 0, 0.5, negIb)
            prep_graph(1)
            square_into(ah2, t2, 1, 0.5, negIb)

            # ---- gamma tiles (unnormalized, bf16), nnr = -(1/||C||)/N, and
            # c1 = (1 - rnorm*gamma00)/N folded from the u-row.  Emitted after
            # the squarings: only needed by the w stage.
            gam = {}
            nnr = sb.tile([1, 1], dt, tag="nnr", name="nnr")
            c1s = sb.tile([1, 1], dt, tag="c1s", name="c1s")
            csq = sb.tile([NF, 1], dt, tag="csq", name="csq")
            nc.vector.tensor_mul(csq, cvec, cvec)
            ps1 = psv.tile([S, N], dt, tag="cr", name="cr")[:1, :1]
            nc.tensor.matmul(ps1, lhsT=csq, rhs=ones11, start=True, stop=True)
            snorm = sb.tile([1, 1], dt, tag="snorm", name="snorm")
            nc.scalar.sqrt(snorm, ps1)
            rnorm = sb.tile([1, 1], dt, tag="rnorm", name="rnorm")
            nc.vector.reciprocal(rnorm, snorm)
            nc.vector.tensor_scalar_mul(nnr, rnorm, -1.0 / N)
            # slices of the 12 flat coefficients: [c00 | c01 | c02 c03 | q1 | q2]
            gam00f = sb.tile([1, 1], dt, tag="gam00f", name="gam00f")
            for key, lo, hi in (("c00", 0, 1), ("c01", 1, 2), ("c023", 2, 4),
                                ("q1", 4, 8)):
                psq = psv.tile([S, N], dt, tag="cr", name="cr")[:hi - lo, :1]
                nc.tensor.matmul(psq, lhsT=gmat[:, lo:hi], rhs=cvec,
                                 start=True, stop=True)
                if key == "c00":
                    nc.vector.tensor_copy(gam00f, psq)
                else:
                    gam[key] = sb.tile([hi - lo, 1], dtb, tag=f"gam_{key}",
                                       name=f"gam_{key}")
                    nc.vector.tensor_copy(gam[key], psq)
            tt = sb.tile([1, 1], dt, tag="tt", name="tt")
            nc.vector.tensor_mul(tt, rnorm, gam00f)
            nc.vector.tensor_scalar(c1s, tt, -1.0 / N, 1.0 / N, Alu.mult, Alu.add)

            # ---- baby vectors + giant chain (bf16 storage, fp32 PSUM)
            gcol = {}
            for g in range(GPC):
                gcol[g] = sb.tile([P, NCH, S], dtb, tag=f"gc{g}", name=f"gc{g}")
                nc.gpsimd.memset(gcol[g][:, :, 0:1], 1.0)

            # PSUM evictions alternate engines per graph so the two
            # graphs' chains drain in parallel (each PSUM-touching DVE/ACT op
            # costs ~150-700ns serially on its engine)
            def ev_copy(g, out, in_):
                nc.vector.tensor_copy(out, in_)

            r1 = {}
            r23 = {}
            z1row = {}
            z2row = {}
            # babies pass 1: g1 = (ah2 @ 1)/2
            onesb = sb.tile([P, 1], dtb, tag="onesb", name="onesb")
            nc.vector.memset(onesb, 1.0)
            for g in range(GPC):
                r1[g] = sb.tile([1, N], dtb, tag=f"r1{g}", name=f"r1{g}")
                ps = psv.tile([S, N], dt, tag="cr", name="cr")[:1, :]
                for kk in range(NCH):
                    nc.tensor.matmul(ps, lhsT=onesb, rhs=ah2[g][:, kk, :],
                                     start=(kk == 0), stop=(kk == NCH - 1))
                ev_copy(g, r1[g], ps)
            # transpose g1 row -> gcol col 1 (stride-2 slots keep PSUM 4B-aligned)
            for g in range(GPC):
                pst = psv.tile([P, NCH * S], dtb, tag="tp", name="tp")[:, :NCH * 2]
                for kk in range(NCH):
                    nc.tensor.transpose(pst[:, kk * 2:kk * 2 + 1],
                                        r1[g][:, kk * P:(kk + 1) * P], identb[:1, :1])
                ev_copy(g, gcol[g][:, :, 1:2],
                        pst.rearrange("p (c two) -> p c two", two=2)[:, :, 0:1])
            # babies pass 2: stream t2 with stationary [u, g1]:
            #   row0 = T2@u = g2 ; row1 = T2@T1@u = h3 (raw; gamma absorbs)
            for g in range(GPC):
                r23[g] = sb.tile([2, N], dtb, tag=f"r23{g}", name=f"r23{g}")
                ps = psv.tile([S, N], dt, tag="cr", name="cr")[:2, :]
                for kk in range(NCH):
                    nc.tensor.matmul(ps, lhsT=gcol[g][:, kk, 0:2], rhs=t2[g][:, kk, :],
                                     start=(kk == 0), stop=(kk == NCH - 1))
                ev_copy(g, r23[g], ps)
            # transpose g2,g3 rows -> gcol cols 2,3
            for g in range(GPC):
                pst = psv.tile([P, NCH * S], dtb, tag="tp", name="tp")[:, :NCH * 2]
                for kk in range(NCH):
                    nc.tensor.transpose(pst[:, kk * 2:(kk + 1) * 2],
                                        r23[g][:, kk * P:(kk + 1) * P], identb[:2, :2])
                ev_copy(g, gcol[g][:, :, 2:4],
                        pst.rearrange("p (c s) -> p c s", s=2))

            # chain: z_raw = T2^2 @ G via two t2 passes (T4 is never built:
            # with MQ=2 it would cost a 16-matmul squaring for one 4-matmul use)
            yrow = {}
            ycol = {}
            for g in range(GPC):
                yrow[g] = sb.tile([S, N], dtb, tag=f"yr{g}", name=f"yr{g}")
                ps = psv.tile([S, N], dt, tag="cr", name="cr")
                for kk in range(NCH):
                    nc.tensor.matmul(ps, lhsT=gcol[g][:, kk, :], rhs=t2[g][:, kk, :],
                                     start=(kk == 0), stop=(kk == NCH - 1))
                ev_copy(g, yrow[g], ps)
            for g in range(GPC):
                ycol[g] = sb.tile([P, NCH, S], dtb, tag=f"yc{g}", name=f"yc{g}")
                pst = psv.tile([P, NCH * S], dtb, tag="tp", name="tp")
                for kk in range(NCH):
                    nc.tensor.transpose(pst[:, kk * S:(kk + 1) * S],
                                        yrow[g][:, kk * P:(kk + 1) * P], identb[:S, :S])
                ev_copy(g, ycol[g].rearrange("p c s -> p (c s)"), pst)
            for g in range(GPC):
                z1row[g] = sb.tile([S, N], dtb, tag=f"z1r{g}", name=f"z1r{g}")
                ps = psv.tile([S, N], dt, tag="cr", name="cr")
                for kk in range(NCH):
                    nc.tensor.matmul(ps, lhsT=ycol[g][:, kk, :], rhs=t2[g][:, kk, :],
                                     start=(kk == 0), stop=(kk == NCH - 1))
                ev_copy(g, z1row[g], ps)
            #     w = c01*g1 + c023^T r23 + q1^T Z1
            #     v = c1s + nnr*w ; emb = v^T X (bf16)
            vrow = {}
            vcol = {}
            for g in range(GPC):
                wps = psv.tile([S, N], dt, tag="cr", name="cr")[:1, :]
                nc.tensor.matmul(wps, lhsT=gam["c01"], rhs=r1[g],
                                 start=True, stop=False, skip_group_check=True)
                nc.tensor.matmul(wps, lhsT=gam["c023"], rhs=r23[g],
                                 start=False, stop=False, skip_group_check=True)
                nc.tensor.matmul(wps, lhsT=gam["q1"], rhs=z1row[g],
                                 start=False, stop=True, skip_group_check=True)
                vrow[g] = sb.tile([1, N], dtb, tag=f"vrow{g}", name=f"vrow{g}")
                nc.scalar.activation(vrow[g], wps,
                                     mybir.ActivationFunctionType.Identity,
                                     bias=c1s[:, 0:1], scale=nnr[:, 0:1])
            for g in range(GPC):
                pst = psv.tile([P, NCH * S], dtb, tag="tp", name="tp")[:, :NCH * 2]
                for kk in range(NCH):
                    nc.tensor.transpose(pst[:, kk * 2:kk * 2 + 1],
                                        vrow[g][:, kk * P:(kk + 1) * P], identb[:1, :1])
                vcol[g] = sb.tile([P, NCH], dtb, tag=f"vc{g}", name=f"vc{g}")
                ev_copy(g, vcol[g],
                        pst.rearrange("p (c two) -> p c two", two=2)[:, :, 0])
            for g in range(GPC):
                pse = psv.tile([S, N], dt, tag="cr", name="cr")[:1, :F]
                for kk in range(NCH):
                    nc.tensor.matmul(pse, lhsT=vcol[g][:, kk:kk + 1],
                                     rhs=xs[g][:, kk, :],
                                     start=(kk == 0), stop=(kk == NCH - 1))
                erow = sb.tile([1, F], dt, tag=f"erow{g}", name=f"erow{g}")
                ev_copy(g, erow, pse)
                nc.scalar.dma_start(emb_d[g:g + 1, :], erow)


# ---------------------------------------------------------------------------
# host: final loss from embeddings (float64; same bookkeeping the reference
# does on the host with numpy: class index construction / product combos)
def final_loss(emb, C, y):
    from itertools import product as _product
    e = emb.astype(np.float64)
    sq = (e * e).sum(1)
    D2 = sq[:, None] + sq[None, :] - 2 * e @ e.T
    D = np.sqrt(np.maximum(D2, 0.0))
    np.fill_diagonal(D, 0.0)
    y = np.asarray(y)
    class_idx = [np.nonzero(y == i)[0] for i in range(K)]
    neg = np.array(list(_product(*class_idx)))
    h1 = -sum(D[np.ix_(cb, cb)].mean() for cb in neg)
    h2 = sum(D[np.ix_(ci, ci)].mean() for ci in class_idx)
    beta = neg.shape[0] / K
    C64 = np.asarray(C, np.float64)
    dims = np.sqrt(float(C64.shape[0]))
    l1 = np.abs(C64).sum(0)
    l2 = np.sqrt((C64 * C64).sum(0))
    sparsity = np.mean((dims - l1 / l2) / (dims - 1))
    return sparsity + h2 + h1 / beta


# ---------------------------------------------------------------------------
_COMPILED = {}


def _get_nc():
    if "nc" in _COMPILED:
        return _COMPILED["nc"]
    import concourse.mybir as mybir
    import concourse.tile as tile
    from concourse import bacc

    dt = mybir.dt.float32
    nc = bacc.Bacc("TRN2", target_bir_lowering=False, debug=False)
    adj_d = nc.dram_tensor("adj", [GPC, P, NCH * N], mybir.dt.bfloat16,
                           kind="ExternalInput").ap()
    x_d = nc.dram_tensor("x", [GPC, P, NCH * F], mybir.dt.bfloat16,
                         kind="ExternalInput").ap()
    c_d = nc.dram_tensor("cvec", [NF, 1], dt, kind="ExternalInput").ap()
    g_d = nc.dram_tensor("gmat", [NF, NG], dt, kind="ExternalInput").ap()
    emb_d = nc.dram_tensor("emb", [GPC, F], dt, kind="ExternalOutput").ap()

    with tile.TileContext(nc) as tc:
        build_device_kernel(tc, emb_d, (adj_d, x_d, c_d, g_d))
    nc.compile()

    _COMPILED["nc"] = nc
    return nc


def kernel(adj, x, C, y):
    global LAST_EXEC_NS, LAST_RESULTS
    from concourse.bass_utils import run_bass_kernel_spmd
    import ml_dtypes

    # adjacency ships as bf16 with entries 2.0 (exact): folds the 2*Ahat
    # factor into the mask multiply; x tolerates bf16 (the emb mean averages
    # the rounding noise far below the accuracy gate).  Both are pre-arranged
    # partition-major ([g, p, chunk*inner]) so device DMAs are contiguous.
    adj = (np.asarray(adj, np.float32) * 2.0).astype(ml_dtypes.bfloat16)
    adj = np.ascontiguousarray(
        adj.reshape(G, NCH, P, N).transpose(0, 2, 1, 3).reshape(G, P, NCH * N))
    x = np.asarray(x, np.float32).astype(ml_dtypes.bfloat16)
    x = np.ascontiguousarray(
        x.reshape(G, NCH, P, F).transpose(0, 2, 1, 3).reshape(G, P, NCH * F))
    C = np.ascontiguousarray(np.asarray(C, np.float32))

    nc = _get_nc()
    in_maps = []
    for c in range(NCORES):
        in_maps.append({
            "adj": adj[c * GPC:(c + 1) * GPC],
            "x": x[c * GPC:(c + 1) * GPC],
            "cvec": C,
            "gmat": GAMMA_MAT,
        })
    import time as _time
    for attempt in range(3):
        try:
            res = run_bass_kernel_spmd(nc, in_maps, core_ids=list(range(NCORES)), trace=TRACE)
            break
        except Exception:
            # transient device errors (e.g. NRT_EXEC_UNIT_UNRECOVERABLE from a
            # previously killed process) clear after a moment
            if attempt == 2:
                raise
            _time.sleep(2.0)
    LAST_EXEC_NS = res.exec_time_ns
    LAST_RESULTS = res
    emb = np.concatenate([res.results[c]["emb"] for c in range(NCORES)], axis=0)
    loss = final_loss(emb, C, y)
    return np.float32(loss)
